# revision 33
# baseline (speedup 1.0000x reference)
"""Trainium2 Bass kernel for AdversarialLogLikelihoodLossLayer.

Per (b,t): negative log-likelihood of a C=40-dim Gaussian
    nll = 0.5*(d^T Sigma^-1 d + logdet Sigma + C*log(2pi)),  d = y_true - mu
summed over T, meaned over B -> scalar.

Algorithm: batched bordered LDL^T (no sqrt, no pivoting; Sigma is SPD and
well-conditioned). Per problem form M = [Sigma | d] (40x41; the d border
column replaces mu in-place after one subtract; the border row is never
materialized). 40 rank-1 Schur eliminations give pivots p_j with
logdet = sum_j log p_j, and since row j is final after step j, the end-state
border column holds w_j[40], so q = d^T Sigma^-1 d = sum_j M[j,40]^2 / p_j
using the saved pivot reciprocals.

Layout (the key trick): 128 problems across SBUF partitions x G=16 matrices
packed G-INNERMOST -- A[128, 40, 41, G] fp16, i.e. element (i,k) of all 16
matrices adjacent. Every DVE operand (the two stride-0 broadcast operands of
the outer product, the subtract, and the u = w/p scale) then has innermost
stride 1 over g with 16 contiguous fp16 elements, which qualifies ALL of
them for the DVE 2x_1p packed mode (the RTL condition is on the innermost
dim only: 2-byte dtype, step +-1, >=2 elements). In the older g-major layout
the broadcast operand had innermost stride 0, pinning the product pass to
1x; a prior iteration fixed that by materializing the broadcast densely on
the Scalar engine (803us -> 637us) before this layout made the ACT bridge
unnecessary (637us -> 567us, removing its cross-engine handshake overhead
too -- HW-vs-costmodel gap shrank from ~31us to ~10us/tile). The host
pre-transposes y_pred into the interleaved layout (to_ginner) so the SWDGE
cast DMA stays dense on both sides, and folds d = y_true - mu into the
border column there (y_true never reaches the device; removes one DMA
stream and the head-of-tile border ops). fp16 inputs are cast on-device by
the SWDGE DMA (loss rel err ~3e-5).

Each step's update runs as DVE tensor_tensor pairs covering only the upper
trapezoid + border column -- the strictly-lower triangle is never read by
later steps, so it is skipped via row-blocks whose columns start at the
block's first row. Block bounds come from an exact DP with per-block
overhead lambda=10 element-equivalents (~2 ops x ~50ns marginal, measured
via CoreSim which tracks HW deltas well). The next step's reciprocal and u
are issued right after the first row-block of the current step (row j+1 is
final then), with double-buffered u. Pivot logs are summed in one ScalarE
Ln+accum op. CoreSim (no_exec) puts DVE occupancy at ~91% with the stream
at the 2x floor (~204us/tile) + ~34us/tile op overhead; remaining levers
are small.

Measured dead ends (do not revisit without new evidence): GPSIMD offload of
block pairs or whole tiles (GPSIMD's SBUF port is physically shared with
the DVE -- the "POOL slot" -- zero overlap, mixed schedules run at the
serial sum); SWDGE accumulate-DMA for the subtract (per-DMA latency on the
40-step chain, 3x slower); u scaled via an ACT-materialized reciprocal
(two extra cross-engine hops on the per-step critical chain, +53us);
stripping same-engine DVE waits (silently corrupts results); coarse DP
bounds when ACT was in the loop (slack elements then cost ACT time too);
DMA_TAIL_FRAC accum-DMA offload of tail-block subtracts (+100us/core on HW
at 0.35 despite CoreSim predicting -29us -- the cost model underestimates
SWDGE accum latency; trust CoreSim for DVE scheduling deltas, NOT for
SWDGE costs).
Considered and rejected: custom fused DVE ops (always 1x -- now strictly
worse than the 2x stock-op pair); TensorE (cannot batch per-partition
independent tiny matmuls); log/Neumann series for logdet/solve (needs
per-problem matmuls anyway).

Data parallel over 8 NeuronCores: 32000 problems -> 4000/core, padded to
4096 = 2 packed tiles with identity problems (which contribute exactly 0);
per-partition partial sums are reduced on the host in float64.
"""

import sys
from contextlib import ExitStack

import numpy as np

sys.path.insert(0, "/opt/trn_rl_repo")

import concourse.bacc as bacc  # noqa: E402
import concourse.bass as bass  # noqa: E402
from concourse import mybir  # noqa: E402
from concourse.bass_utils import run_bass_kernel_spmd  # noqa: E402
from concourse.tile import TileContext  # noqa: E402

B, T, C = 64, 500, 40
CP1 = C + 1
N_CORES = 8
NPROB = B * T
PER_CORE = NPROB // N_CORES  # 4000
PAD = 4096                   # per-core padded problem count
NTILES_FULL = PAD // 128     # 32

F32 = mybir.dt.float32
F16 = mybir.dt.float16
OP = mybir.AluOpType
DT_A = F16      # dtype of the working matrix / products (F16 -> 2x subtract)
CAST_VIA_ACT = False  # False: SWDGE cast DMA; True: HWDGE + ACT copy-cast
LOG_2PI = float(np.log(2.0 * np.pi))
SPLIT_THRESHOLDS = (3, 5, 8, 11, 15, 19, 23, 27, 31, 35, 39)
SPLIT_LAMBDA = 10.0
USE_DP_BOUNDS = True
ABUFS = 4
PBUFS = 2
SBUFS = 6
U_ON_GPSIMD = False
U_ON_ACT = False
STRIP_DVE_WAITS = False  # UNSAFE: silently corrupts results (races); keep off
SUB_VIA_DMA = False
GP_FRAC = 0.0   # fraction of block elements routed to GPSIMD (0 = all DVE)
# Legacy g-major path only (_emit_tile_packed): materialize the broadcast
# u-operand densely on the Scalar engine so the DVE product gets 2x_1p.
# The g-innermost layout (_emit_tile_ginner, used by kernel()) makes every
# operand 2x-eligible directly and does not need ACT at all.
USE_ACT_UM = True
UMBUFS = 4
# g-inner path: offload the subtract of the bottom DMA_TAIL_FRAC rows of each
# step's update to a SWDGE accumulate-DMA (Pool engine + DMA are ~idle; in the
# g-inner layout the tail block is one 3-dim AP with (m-rs)*G contiguous inner
# elements). Requires u negated (CCE supports add, not subtract): RA is
# negated in place after each recip, every DVE block update becomes add, and
# the final combine compensates the sign of q.
# Measured on HW: +100us/core at 0.35/14 (sim predicted -29us; the cost model
# underestimates SWDGE accum latency -- same verdict as the older wholesale
# experiment). Keep 0.
DMA_TAIL_FRAC = 0.0
DMA_MIN_M = 14   # only offload steps with m >= this (late steps lack slack)
LOAD_CHUNKS = 8  # split each A load into row-chunks (pipelines fill w/ compute)


_BOUNDS_CACHE: dict = {}


def _opt_bounds(mr: int, m: int):
    """Optimal row-block boundaries covering rows [0, mr) of the upper
    trapezoid, where a block [r, e) costs (e-r)*(m-r) streamed elements plus
    SPLIT_LAMBDA element-equivalents of per-op-pair overhead. Exact DP."""
    key = (mr, m, SPLIT_LAMBDA)
    if key in _BOUNDS_CACHE:
        return _BOUNDS_CACHE[key]
    lam = SPLIT_LAMBDA
    INF = float("inf")
    dp = [INF] * (mr + 1)
    nxt = [0] * (mr + 1)
    dp[mr] = 0.0
    for r in range(mr - 1, -1, -1):
        for e in range(r + 1, mr + 1):
            c = (e - r) * (m - r) + lam + dp[e]
            if c < dp[r]:
                dp[r], nxt[r] = c, e
    bounds = [0]
    r = 0
    while r < mr:
        r = nxt[r]
        bounds.append(r)
    _BOUNDS_CACHE[key] = bounds
    return bounds


def _strip_same_engine_dve_waits(nc):
    """Drop DVE-semaphore waits from instructions executing on the DVE.

    The DVE executes its queue strictly in order and flushes its pipeline
    (DRAIN) after every op before the next can issue, so RAW/WAR between two
    DVE instructions is enforced by hardware; the semaphore wait Tile emits
    for them only adds issue latency. Cross-engine waits are preserved.
    """
    n = 0
    for blk in nc.m.functions[0].blocks:
        for inst in blk.instructions:
            si = inst.sync_info
            if si is None or not si.on_wait:
                continue
            if str(getattr(inst, "engine", "")) != "EngineType.DVE":
                continue
            kept = [w for w in si.on_wait
                    if not (w.ant_name or "").startswith("DVE")]
            if len(kept) != len(si.on_wait):
                n += len(si.on_wait) - len(kept)
                inst.sync_info = mybir.SyncInfo(on_wait=kept,
                                                on_update=list(si.on_update))
    return n


def _emit_tile(nc, pools, yp, yt, vout, t):
    """Emit the full processing of one 128-problem tile."""
    apool, ppool, spool, _gpool, _upool = pools
    lo = t * 128
    A = apool.tile([128, CP1, CP1], F32, tag="A")
    D = spool.tile([128, C], F32, tag="D")
    # Sigma rows + mu land directly in M[0:40, :]: y_pred row-major
    # [40,41] matches M's first 40 rows, mu in column 40.
    nc.sync.dma_start(
        out=A[:, 0:C, :],
        in_=yp[lo : lo + 128, :].rearrange("p (i k) -> p i k", i=C),
    )
    nc.sync.dma_start(out=D[:], in_=yt[lo : lo + 128, :])
    # d = y_true - mu
    nc.vector.tensor_tensor(out=D[:], in0=D[:], in1=A[:, 0:C, C], op=OP.subtract)
    nc.vector.tensor_copy(out=A[:, 0:C, C], in_=D[:])   # border column
    nc.vector.tensor_copy(out=A[:, C, 0:C], in_=D[:])   # border row
    nc.vector.memset(A[:, C, C : C + 1], 0.0)           # corner

    for j in range(C):
        m = CP1 - 1 - j  # trailing block size
        R = spool.tile([128, 1], F32, tag="R")
        nc.vector.reciprocal(out=R[:], in_=A[:, j, j : j + 1])
        row = A[:, j, j + 1 : CP1]                      # [128, m] pivot row
        v_i = row[:, :, None].broadcast_to([128, m, m])  # w[i] over (i,k)
        v_k = row[:, None, :].broadcast_to([128, m, m])  # w[k] over (i,k)
        Pt = ppool.tile([128, m, m], F32, tag="P")
        nc.vector.scalar_tensor_tensor(
            out=Pt[:], in0=v_i, scalar=R[:], in1=v_k, op0=OP.mult, op1=OP.mult
        )
        nc.vector.tensor_tensor(
            out=A[:, j + 1 :, j + 1 :], in0=A[:, j + 1 :, j + 1 :], in1=Pt[:],
            op=OP.subtract,
        )

    # v = sum_j log(pivot_j) - corner   (corner = -d^T Sigma^-1 d)
    a = A[:]
    diag = bass.AP(tensor=a.tensor, offset=a.offset, ap=[a.ap[0], [CP1 + 1, C]])
    LOGT = spool.tile([128, C], F32, tag="LOG")
    S = spool.tile([128, 1], F32, tag="S")
    nc.scalar.activation(
        out=LOGT[:], in_=diag, func=mybir.ActivationFunctionType.Ln,
        accum_out=S[:],
    )
    V = spool.tile([128, 1], F32, tag="V")
    nc.vector.tensor_tensor(out=V[:], in0=S[:], in1=A[:, C, C : C + 1], op=OP.subtract)
    nc.sync.dma_start(out=vout[lo : lo + 128, :], in_=V[:])


def _make_pools(tc, ctx, G: int = 1):
    per_buf = G * C * CP1 * (2 if DT_A is F16 else 4)
    if DT_A is F16 and CAST_VIA_ACT:
        per_buf += G * C * CP1 * 4  # f32 staging tile shares the pool buf
    if per_buf <= 30 * 1024:
        abufs = ABUFS
    else:
        abufs = 2
    sbufs = SBUFS if G <= 4 else 3
    apool = ctx.enter_context(tc.tile_pool(name="A", bufs=abufs))
    ppool = ctx.enter_context(tc.tile_pool(name="P", bufs=PBUFS))
    gpool = ctx.enter_context(tc.tile_pool(name="gpP", bufs=PBUFS))
    spool = ctx.enter_context(tc.tile_pool(name="small", bufs=sbufs))
    upool = ctx.enter_context(tc.tile_pool(name="UM", bufs=UMBUFS))
    return apool, ppool, spool, gpool, upool


def _gp_split(bounds, m: int):
    """First block index of the suffix routed to GPSIMD: targets GP_FRAC of
    this step's streamed elements; block 0 always stays on DVE (it carries
    the next step's pivot row)."""
    els = [
        (bounds[b + 1] - bounds[b]) * (m - bounds[b])
        for b in range(len(bounds) - 1)
    ]
    budget = GP_FRAC * sum(els)
    acc, start = 0.0, len(els)
    for b in range(len(els) - 1, 0, -1):
        if abs(acc + els[b] - budget) >= abs(acc - budget):
            break
        acc += els[b]
        start = b
    return start


def _emit_tile_packed(nc, pools, yp, yt, vout, t, G, big_eng=None):
    """One tile = G*128 problems: G matrices packed along the free dim of
    each partition. Outputs one partially-summed value per partition.

    The border ROW (d^T) is never materialized: at step j the update only
    writes rows j+1..39 x cols j+1..40. Row j is final after step j, so the
    end-state matrix holds every pivot row; with the saved reciprocals the
    quadratic form is q = sum_j A[j,40]^2 / p_j.
    """
    apool, ppool, spool, gpool, upool = pools
    big = big_eng if big_eng is not None else nc.vector
    dt = DT_A
    lo = t * G * 128
    A = apool.tile([128, G, C, CP1], dt, tag="A")           # rows 0..39 only
    yp_ap = yp[lo : lo + G * 128, :].rearrange("(g p) (i k) -> p g i k", g=G, i=C)
    if dt is F32:
        nc.sync.dma_start(out=A[:], in_=yp_ap)
    elif CAST_VIA_ACT:
        AS = apool.tile([128, G, C, CP1], F32, tag="AS")
        nc.sync.dma_start(out=AS[:], in_=yp_ap)
        nc.scalar.activation(
            out=A[:], in_=AS[:], func=mybir.ActivationFunctionType.Copy)
    else:
        nc.gpsimd.dma_start(out=A[:], in_=yp_ap)           # SWDGE f32->f16 cast
    D = spool.tile([128, G, C], dt, tag="D")
    RA = spool.tile([128, G, C], dt, tag="RA")              # 1/pivot per step
    yt_ap = yt[lo : lo + G * 128, :].rearrange("(g p) c -> p g c", g=G)
    if dt is F32:
        nc.sync.dma_start(out=D[:], in_=yt_ap)
    else:
        nc.gpsimd.dma_start(out=D[:], in_=yt_ap)            # SWDGE f32->f16 cast
    # border column: d = y_true - mu  (mu is already in column 40)
    nc.vector.tensor_tensor(out=D[:], in0=D[:], in1=A[:, :, 0:C, C], op=OP.subtract)
    nc.vector.tensor_copy(out=A[:, :, 0:C, C], in_=D[:])

    U0 = spool.tile([128, G, C], dt, tag="U0")
    U1 = spool.tile([128, G, C], dt, tag="U1")
    Us = [U0, U1]

    def emit_recip_u(j):
        """recip_j, then u_j = row_j / p_j (into Us[j%2]); u skipped on the
        last step (no trailing rows)."""
        with nc.allow_low_precision(reason="per-pivot reciprocal, not an accum"):
            nc.vector.reciprocal(out=RA[:, :, j : j + 1], in_=A[:, :, j, j : j + 1])
        if j == C - 1:
            return
        m = C - j
        rj = A[:, :, j, j + 1 : CP1]
        if U_ON_ACT:
            for g in range(G):
                nc.scalar.activation(
                    out=Us[j % 2][:, g, 0:m], in_=A[:, g, j, j + 1 : CP1],
                    func=mybir.ActivationFunctionType.Copy,
                    scale=RA[:, g, j : j + 1],
                )
        else:
            ueng = nc.gpsimd if U_ON_GPSIMD else big
            ueng.tensor_tensor(
                out=Us[j % 2][:, :, 0:m], in0=rj,
                in1=RA[:, :, j : j + 1].broadcast_to([128, G, m]), op=OP.mult,
            )

    emit_recip_u(0)
    for j in range(C - 1):
        m = C - j  # trailing columns j+1..40 (incl. border col) = m
        U = Us[j % 2]
        row = A[:, :, j, j + 1 : CP1]                       # [128, G, m]
        mr = m - 1                                          # rows j+1..39
        # Only entries (i, k>=i) plus the border column are ever read later;
        # cover the upper trapezoid with row-blocks whose columns start at
        # the block's first row (bounding rectangles).
        if USE_DP_BOUNDS:
            bounds = _opt_bounds(mr, m)
        else:
            nb = 1 + sum(mr >= th for th in SPLIT_THRESHOLDS)
            bounds = [rs * mr // nb for rs in range(nb)] + [mr]
        gp_start = (
            _gp_split(bounds, m)
            if (GP_FRAC > 0 and big_eng is None and not SUB_VIA_DMA)
            else len(bounds)
        )
        for b in range(len(bounds) - 1):
            rs, re = bounds[b], bounds[b + 1]
            nrows = re - rs
            v_i = U[:, :, rs:re, None].broadcast_to([128, G, nrows, m - rs])
            v_k = row[:, :, None, rs:m].broadcast_to([128, G, nrows, m - rs])
            if SUB_VIA_DMA:
                # Pt holds -(w/p) (x) w in a full-row-width (41) padded tile
                # so the accumulate DMA collapses to a 3-dim AP. Columns left
                # of the block's start hold stale garbage that lands in
                # strictly-lower cells of A, which are never read.
                Pt = ppool.tile([128, G, nrows, CP1], dt, tag="P")
                big.tensor_tensor(
                    out=Pt[:, :, :, j + 1 + rs : CP1], in0=v_i, in1=v_k, op=OP.mult
                )
                blk_full = A[:, :, j + 1 + rs : j + 1 + re, :]
                nc.gpsimd.dma_start(out=blk_full, in_=Pt[:], accum_op=OP.add)
            else:
                on_gp = b >= gp_start
                eng = nc.gpsimd if on_gp else big
                pool = gpool if (on_gp or big_eng is not None) else ppool
                Pt = pool.tile([128, G, nrows, m - rs], dt, tag="P")
                if USE_ACT_UM and not on_gp:
                    UM = upool.tile([128, G, nrows, m - rs], dt, tag="UM")
                    nc.scalar.activation(
                        out=UM[:], in_=v_i,
                        func=mybir.ActivationFunctionType.Copy,
                    )
                    eng.tensor_tensor(out=Pt[:], in0=UM[:], in1=v_k, op=OP.mult)
                else:
                    eng.tensor_tensor(out=Pt[:], in0=v_i, in1=v_k, op=OP.mult)
                blk = A[:, :, j + 1 + rs : j + 1 + re, j + 1 + rs :]
                eng.tensor_tensor(out=blk, in0=blk, in1=Pt[:], op=OP.subtract)
            if b == 0:
                # row j+1 is final: issue the next step's recip + u now so
                # the cross-step chain doesn't wait on this step's tail.
                emit_recip_u(j + 1)

    # per-partition partial sum over g: sum_j log(p_j) + sum_j dcol_j^2/p_j
    a = A[:]
    diag = bass.AP(
        tensor=a.tensor, offset=a.offset,
        ap=[a.ap[0], [C * CP1, G], [CP1 + 1, C]],
    )
    LOGT = spool.tile([128, G, C], dt, tag="LOG")
    S = spool.tile([128, 1], F32, tag="S")
    nc.scalar.activation(
        out=LOGT[:], in_=diag, func=mybir.ActivationFunctionType.Ln,
        accum_out=S[:],
    )
    dcol = A[:, :, 0:C, C]                                  # final border col
    SQ = spool.tile([128, G, C], dt, tag="SQ")
    nc.vector.tensor_tensor(out=SQ[:], in0=dcol, in1=dcol, op=OP.mult)
    Q = spool.tile([128, 1], F32, tag="Q")
    nc.vector.scalar_tensor_tensor(
        out=SQ[:], in0=SQ[:], scalar=1.0, in1=RA[:], op0=OP.mult, op1=OP.mult,
        accum_out=Q[:],
    )
    V = spool.tile([128, 1], F32, tag="V")
    nc.vector.tensor_tensor(out=V[:], in0=S[:], in1=Q[:], op=OP.add)
    nc.sync.dma_start(out=vout[t * 128 : (t + 1) * 128, :], in_=V[:])


def _emit_tile_ginner(nc, pools, yp, vout, t, G):
    """g-innermost variant: one tile = G*128 problems stored interleaved as
    A[128, C, CP1, G] (fp16), i.e. element (i,k) of all G matrices adjacent.

    Every update operand then has innermost stride 1 over g (16 contiguous
    fp16 elements), so the outer-product mult, the subtract, AND the u scale
    all qualify for the DVE 2x_1p packed mode without materializing any
    broadcast operand -- the Scalar engine is not needed at all. The host
    pre-transposes y_pred/y_true to this layout so the load DMA stays dense.
    """
    apool, ppool, spool, _gpool, _upool = pools
    big = nc.vector
    dt = DT_A
    lo = t * 128
    A = apool.tile([128, C, CP1, G], dt, tag="A")
    # border column already holds d = y_true - mu (folded on the host).
    # Load in row-chunks so step-0 blocks start as soon as their rows land
    # (cuts the initial fill gap ~22us -> ~6us; Tile's data deps do the rest).
    rows_per = (C + LOAD_CHUNKS - 1) // LOAD_CHUNKS
    for r0 in range(0, C, rows_per):
        r1 = min(C, r0 + rows_per)
        nc.gpsimd.dma_start(
            out=A[:, r0:r1, :, :],
            in_=yp[lo : lo + 128, r0 * CP1 * G : r1 * CP1 * G].rearrange(
                "p (i k g) -> p i k g", i=r1 - r0, k=CP1))  # SWDGE f32->f16 cast
    RA = spool.tile([128, C, G], dt, tag="RA")            # 1/pivot per step

    U0 = spool.tile([128, C, G], dt, tag="U0")
    U1 = spool.tile([128, C, G], dt, tag="U1")
    Us = [U0, U1]

    neg_u = DMA_TAIL_FRAC > 0
    upd_op = OP.add if neg_u else OP.subtract

    def emit_recip_u(j):
        with nc.allow_low_precision(reason="per-pivot reciprocal, not an accum"):
            nc.vector.reciprocal(out=RA[:, j, :], in_=A[:, j, j, :])
        if neg_u:
            # RA holds -1/p: u comes out negated (updates become adds, the
            # accum-DMA's CCE only has add) and q's sign is fixed at the end.
            nc.vector.tensor_scalar(
                out=RA[:, j, :], in0=RA[:, j, :], scalar1=-1.0, scalar2=None,
                op0=OP.mult)
        if j == C - 1:
            return
        m = C - j
        big.tensor_tensor(
            out=Us[j % 2][:, 0:m, :], in0=A[:, j, j + 1 : CP1, :],
            in1=RA[:, j, None, :].broadcast_to([128, m, G]), op=OP.mult,
        )

    emit_recip_u(0)
    for j in range(C - 1):
        m = C - j
        U = Us[j % 2]
        row = A[:, j, j + 1 : CP1, :]                     # [128, m, G]
        mr = m - 1
        ts = mr  # first row of the DMA-offloaded tail block
        if DMA_TAIL_FRAC > 0 and m >= DMA_MIN_M:
            ts = int(mr * (1.0 - DMA_TAIL_FRAC))
            if mr - ts < 4:
                ts = mr
        if USE_DP_BOUNDS:
            bounds = _opt_bounds(ts, m)
        else:
            nb = 1 + sum(ts >= th for th in SPLIT_THRESHOLDS)
            bounds = [rs * ts // nb for rs in range(nb)] + [ts]
        if ts < mr:
            bounds = bounds + [mr]
        for b in range(len(bounds) - 1):
            rs, re = bounds[b], bounds[b + 1]
            nrows = re - rs
            v_i = U[:, rs:re, None, :].broadcast_to([128, nrows, m - rs, G])
            v_k = row[:, None, rs:m, :].broadcast_to([128, nrows, m - rs, G])
            Pt = ppool.tile([128, nrows, m - rs, G], dt, tag="P")
            big.tensor_tensor(out=Pt[:], in0=v_i, in1=v_k, op=OP.mult)
            blk = A[:, j + 1 + rs : j + 1 + re, j + 1 + rs :, :]
            if rs >= ts:
                nc.gpsimd.dma_start(out=blk, in_=Pt[:], accum_op=OP.add)
            else:
                big.tensor_tensor(out=blk, in0=blk, in1=Pt[:], op=upd_op)
            if b == 0:
                emit_recip_u(j + 1)

    # per-partition partial sum: sum_j log(p_j) + sum_j dcol_j^2 / p_j
    a = A[:]
    diag = bass.AP(
        tensor=a.tensor, offset=a.offset,
        ap=[a.ap[0], [(CP1 + 1) * G, C], [1, G]],
    )
    LOGT = spool.tile([128, C, G], dt, tag="LOG")
    S = spool.tile([128, 1], F32, tag="S")
    nc.scalar.activation(
        out=LOGT[:], in_=diag, func=mybir.ActivationFunctionType.Ln,
        accum_out=S[:],
    )
    dcol = A[:, 0:C, C, :]                                # final border col
    SQ = spool.tile([128, C, G], dt, tag="SQ")
    nc.vector.tensor_tensor(out=SQ[:], in0=dcol, in1=dcol, op=OP.mult)
    Q = spool.tile([128, 1], F32, tag="Q")
    nc.vector.scalar_tensor_tensor(
        out=SQ[:], in0=SQ[:], scalar=1.0, in1=RA[:], op0=OP.mult, op1=OP.mult,
        accum_out=Q[:],
    )
    V = spool.tile([128, 1], F32, tag="V")
    # with u negated, RA holds -1/p so Q = -q: compensate here
    nc.vector.tensor_tensor(
        out=V[:], in0=S[:], in1=Q[:], op=OP.subtract if neg_u else OP.add)
    nc.sync.dma_start(out=vout[t * 128 : (t + 1) * 128, :], in_=V[:])


def build3(ntiles: int, G: int) -> bass.Bass:
    """g-innermost packed variant (host pre-transposed inputs)."""
    nc = bacc.Bacc("TRN2", target_bir_lowering=False)
    yp = nc.dram_tensor("y_pred", [ntiles * 128, C * CP1 * G], F32,
                        kind="ExternalInput")
    vout = nc.dram_tensor("v_out", [ntiles * 128, 1], F32, kind="ExternalOutput")
    with TileContext(nc) as tc, ExitStack() as ctx:
        pools = _make_pools(tc, ctx, G)
        for t in range(ntiles):
            _emit_tile_ginner(nc, pools, yp, vout, t, G)
    if not nc.is_finalized():
        nc.finalize()
    return nc


def build_loop3(body_tiles: int, reps: int, G: int) -> bass.Bass:
    nc = bacc.Bacc("TRN2", target_bir_lowering=False)
    yp = nc.dram_tensor("y_pred", [body_tiles * 128, C * CP1 * G], F32,
                        kind="ExternalInput")
    vout = nc.dram_tensor("v_out", [body_tiles * 128, 1], F32,
                          kind="ExternalOutput")
    with TileContext(nc) as tc, ExitStack() as ctx:
        pools = _make_pools(tc, ctx, G)

        def body(i, unroll=1):
            for t in range(body_tiles):
                _emit_tile_ginner(nc, pools, yp, vout, t, G)

        with tc.For_i(0, reps, 1) as i:
            body(i)
    if not nc.is_finalized():
        nc.finalize()
    return nc


BORDER_IDX = np.arange(C) * CP1 + C


def to_ginner(ypf: np.ndarray, ytf: np.ndarray, ntiles: int, G: int):
    """Host-side relayout: per tile, problems (g, p) -> partition p holds the
    G matrices interleaved element-wise: row p = y_pred[(g,p), i, k] laid out
    as (i, k, g). The border column is replaced by d = y_true - mu so the
    device needs neither y_true nor the border subtract."""
    n = ntiles * G * 128
    yp = ypf[:n].copy()
    yp[:, BORDER_IDX] = ytf[:n] - yp[:, BORDER_IDX]
    yp = yp.reshape(ntiles, G, 128, C * CP1)
    yp = yp.transpose(0, 2, 3, 1).reshape(ntiles * 128, C * CP1 * G)
    return np.ascontiguousarray(yp)


def build(ntiles: int = NTILES_FULL) -> bass.Bass:
    nprob = ntiles * 128
    nc = bacc.Bacc("TRN2", target_bir_lowering=False)
    yp = nc.dram_tensor("y_pred", [nprob, C * CP1], F32, kind="ExternalInput")
    yt = nc.dram_tensor("y_true", [nprob, C], F32, kind="ExternalInput")
    vout = nc.dram_tensor("v_out", [nprob, 1], F32, kind="ExternalOutput")

    with TileContext(nc) as tc, ExitStack() as ctx:
        pools = _make_pools(tc, ctx)
        for t in range(ntiles):
            _emit_tile(nc, pools, yp, yt, vout, t)
    if not nc.is_finalized():
        nc.finalize()
    return nc


def build_loop(body_tiles: int, reps: int) -> bass.Bass:
    """Timing amplifier: process the same `body_tiles` tiles `reps` times
    inside a For_i loop (static addressing; WAW across reps is fine)."""
    nprob = body_tiles * 128
    nc = bacc.Bacc("TRN2", target_bir_lowering=False)
    yp = nc.dram_tensor("y_pred", [nprob, C * CP1], F32, kind="ExternalInput")
    yt = nc.dram_tensor("y_true", [nprob, C], F32, kind="ExternalInput")
    vout = nc.dram_tensor("v_out", [nprob, 1], F32, kind="ExternalOutput")

    with TileContext(nc) as tc, ExitStack() as ctx:
        pools = _make_pools(tc, ctx)

        def body(i, unroll=1):
            for t in range(body_tiles):
                _emit_tile(nc, pools, yp, yt, vout, t)

        with tc.For_i(0, reps, 1) as i:
            body(i)
    if not nc.is_finalized():
        nc.finalize()
    return nc


def build2(ntiles: int, G: int) -> bass.Bass:
    """Packed variant: each tile covers G*128 problems."""
    nprob = ntiles * G * 128
    nc = bacc.Bacc("TRN2", target_bir_lowering=False)
    yp = nc.dram_tensor("y_pred", [nprob, C * CP1], F32, kind="ExternalInput")
    yt = nc.dram_tensor("y_true", [nprob, C], F32, kind="ExternalInput")
    vout = nc.dram_tensor("v_out", [ntiles * 128, 1], F32, kind="ExternalOutput")

    with TileContext(nc) as tc, ExitStack() as ctx:
        pools = _make_pools(tc, ctx, G)
        for t in range(ntiles):
            _emit_tile_packed(nc, pools, yp, yt, vout, t, G)
    if STRIP_DVE_WAITS:
        _strip_same_engine_dve_waits(nc)
    if not nc.is_finalized():
        nc.finalize()
    return nc


def build_loop2(body_tiles: int, reps: int, G: int, gp_every: int = 0,
                gp_tiles=()) -> bass.Bass:
    """gp_every=k: every k-th tile runs its big ops on GPSIMD (0 = never).
    gp_tiles: explicit tile indices to run on GPSIMD (overrides gp_every)."""
    nprob = body_tiles * G * 128
    nc = bacc.Bacc("TRN2", target_bir_lowering=False)
    yp = nc.dram_tensor("y_pred", [nprob, C * CP1], F32, kind="ExternalInput")
    yt = nc.dram_tensor("y_true", [nprob, C], F32, kind="ExternalInput")
    vout = nc.dram_tensor("v_out", [body_tiles * 128, 1], F32, kind="ExternalOutput")

    with TileContext(nc) as tc, ExitStack() as ctx:
        pools = _make_pools(tc, ctx, G)

        def body(i, unroll=1):
            for t in range(body_tiles):
                on_gp = (t in gp_tiles) or (gp_every and t % gp_every == gp_every - 1)
                eng = nc.gpsimd if on_gp else None
                _emit_tile_packed(nc, pools, yp, yt, vout, t, G, big_eng=eng)

        with tc.For_i(0, reps, 1) as i:
            body(i)
    if STRIP_DVE_WAITS:
        _strip_same_engine_dve_waits(nc)
    if not nc.is_finalized():
        nc.finalize()
    return nc


_CACHE: dict = {}


def _pad_rows(n_pad: int) -> tuple[np.ndarray, np.ndarray]:
    """Identity problems: Sigma=I, mu=0, y_true=0 -> v contribution exactly 0."""
    row = np.concatenate([np.eye(C, dtype=np.float32), np.zeros((C, 1), np.float32)], axis=1)
    return (
        np.tile(row.reshape(1, -1), (n_pad, 1)),
        np.zeros((n_pad, C), np.float32),
    )


G_PACK = 16
NTILES_PACKED = PAD // (G_PACK * 128)  # 2


def kernel(y_true: np.ndarray, y_pred: np.ndarray) -> np.ndarray:
    # np.asarray also handles jax arrays (device -> host copy)
    ypf = np.ascontiguousarray(
        np.asarray(y_pred, dtype=np.float32).reshape(NPROB, C * CP1))
    ytf = np.ascontiguousarray(
        np.asarray(y_true, dtype=np.float32).reshape(NPROB, C))

    if "nc" not in _CACHE:
        _CACHE["nc"] = build3(NTILES_PACKED, G_PACK)
    nc = _CACHE["nc"]

    pad_p, pad_t = _pad_rows(PAD - PER_CORE)
    in_maps = []
    for c in range(N_CORES):
        sl = slice(c * PER_CORE, (c + 1) * PER_CORE)
        ypg = to_ginner(
            np.concatenate([ypf[sl], pad_p], axis=0),
            np.concatenate([ytf[sl], pad_t], axis=0),
            NTILES_PACKED, G_PACK,
        )
        in_maps.append({"y_pred": ypg})

    # Transient device flakes (observed ~once per dozen runs) can yield NaN;
    # the result is cheap to validate, so retry a couple of times on
    # non-finite output before giving up.
    for _attempt in range(3):
        res = run_bass_kernel_spmd(nc, in_maps, core_ids=list(range(N_CORES)))
        # v_out rows are per-partition partial sums (padding contributes 0)
        v = np.concatenate([r["v_out"][:, 0] for r in res.results])
        loss = 0.5 * float(np.sum(v, dtype=np.float64)) / B + T * 0.5 * C * LOG_2PI
        if np.isfinite(loss):
            break
    return np.float32(loss)



# revision 34
# speedup vs baseline: 1.1874x; 1.1874x over previous
"""Trainium2 Bass kernel for AdversarialLogLikelihoodLossLayer.

Per (b,t): negative log-likelihood of a C=40-dim Gaussian
    nll = 0.5*(d^T Sigma^-1 d + logdet Sigma + C*log(2pi)),  d = y_true - mu
summed over T, meaned over B -> scalar.

Algorithm: batched bordered LDL^T (no sqrt, no pivoting; Sigma is SPD and
well-conditioned). Per problem form M = [Sigma | d] (40x41; the d border
column replaces mu in-place after one subtract; the border row is never
materialized). 40 rank-1 Schur eliminations give pivots p_j with
logdet = sum_j log p_j, and since row j is final after step j, the end-state
border column holds w_j[40], so q = d^T Sigma^-1 d = sum_j M[j,40]^2 / p_j
using the saved pivot reciprocals.

Layout (the key trick): 128 problems across SBUF partitions x G=16 matrices
packed G-INNERMOST -- A[128, 40, 41, G] fp16, i.e. element (i,k) of all 16
matrices adjacent. Every DVE operand (the two stride-0 broadcast operands of
the outer product, the subtract, and the u = w/p scale) then has innermost
stride 1 over g with 16 contiguous fp16 elements, which qualifies ALL of
them for the DVE 2x_1p packed mode (the RTL condition is on the innermost
dim only: 2-byte dtype, step +-1, >=2 elements). In the older g-major layout
the broadcast operand had innermost stride 0, pinning the product pass to
1x; a prior iteration fixed that by materializing the broadcast densely on
the Scalar engine (803us -> 637us) before this layout made the ACT bridge
unnecessary (637us -> 567us, removing its cross-engine handshake overhead
too -- HW-vs-costmodel gap shrank from ~31us to ~10us/tile). The host
pre-transposes y_pred into the interleaved layout (to_ginner) so the SWDGE
cast DMA stays dense on both sides, and folds d = y_true - mu into the
border column there (y_true never reaches the device; removes one DMA
stream and the head-of-tile border ops). fp16 inputs are cast on-device by
the SWDGE DMA (loss rel err ~3e-5).

Each step's update runs as DVE tensor_tensor pairs covering only the upper
trapezoid + border column -- the strictly-lower triangle is never read by
later steps, so it is skipped via row-blocks whose columns start at the
block's first row. Block bounds come from an exact DP with per-block
overhead lambda=10 element-equivalents (~2 ops x ~50ns marginal, measured
via CoreSim which tracks HW deltas well). The next step's reciprocal and u
are issued right after the first row-block of the current step (row j+1 is
final then), with double-buffered u. Pivot logs are summed in one ScalarE
Ln+accum op. CoreSim (no_exec) puts DVE occupancy at ~91% with the stream
at the 2x floor (~204us/tile) + ~34us/tile op overhead; remaining levers
are small.

Measured dead ends (do not revisit without new evidence): GPSIMD offload of
block pairs or whole tiles (GPSIMD's SBUF port is physically shared with
the DVE -- the "POOL slot" -- zero overlap, mixed schedules run at the
serial sum); SWDGE accumulate-DMA for the subtract (per-DMA latency on the
40-step chain, 3x slower); u scaled via an ACT-materialized reciprocal
(two extra cross-engine hops on the per-step critical chain, +53us);
stripping same-engine DVE waits (silently corrupts results); coarse DP
bounds when ACT was in the loop (slack elements then cost ACT time too);
DMA_TAIL_FRAC accum-DMA offload of tail-block subtracts (+100us/core on HW
at 0.35 despite CoreSim predicting -29us -- the cost model underestimates
SWDGE accum latency; trust CoreSim for DVE scheduling deltas, NOT for
SWDGE costs).
Considered and rejected: custom fused DVE ops (always 1x -- now strictly
worse than the 2x stock-op pair); TensorE (cannot batch per-partition
independent tiny matmuls); log/Neumann series for logdet/solve (needs
per-problem matmuls anyway).

Data parallel over 8 NeuronCores: 32000 problems -> 4000/core, padded to
4096 = 2 packed tiles with identity problems (which contribute exactly 0);
per-partition partial sums are reduced on the host in float64.
"""

import sys
from contextlib import ExitStack

import numpy as np

sys.path.insert(0, "/opt/trn_rl_repo")

import concourse.bacc as bacc  # noqa: E402
import concourse.bass as bass  # noqa: E402
from concourse import mybir  # noqa: E402
from concourse.bass_utils import run_bass_kernel_spmd  # noqa: E402
from concourse.tile import TileContext  # noqa: E402

B, T, C = 64, 500, 40
CP1 = C + 1
N_CORES = 8
NPROB = B * T
PER_CORE = NPROB // N_CORES  # 4000
PAD = 4096                   # per-core padded problem count
NTILES_FULL = PAD // 128     # 32

F32 = mybir.dt.float32
F16 = mybir.dt.float16
OP = mybir.AluOpType
DT_A = F16      # dtype of the working matrix / products (F16 -> 2x subtract)
CAST_VIA_ACT = False  # False: SWDGE cast DMA; True: HWDGE + ACT copy-cast
LOG_2PI = float(np.log(2.0 * np.pi))
SPLIT_THRESHOLDS = (3, 5, 8, 11, 15, 19, 23, 27, 31, 35, 39)
SPLIT_LAMBDA = 10.0
USE_DP_BOUNDS = True
ABUFS = 4
PBUFS = 2
SBUFS = 6
U_ON_GPSIMD = False
U_ON_ACT = False
STRIP_DVE_WAITS = False  # UNSAFE: silently corrupts results (races); keep off
SUB_VIA_DMA = False
GP_FRAC = 0.0   # fraction of block elements routed to GPSIMD (0 = all DVE)
# Legacy g-major path only (_emit_tile_packed): materialize the broadcast
# u-operand densely on the Scalar engine so the DVE product gets 2x_1p.
# The g-innermost layout (_emit_tile_ginner, used by kernel()) makes every
# operand 2x-eligible directly and does not need ACT at all.
USE_ACT_UM = True
UMBUFS = 4
# g-inner path: offload the subtract of the bottom DMA_TAIL_FRAC rows of each
# step's update to a SWDGE accumulate-DMA (Pool engine + DMA are ~idle; in the
# g-inner layout the tail block is one 3-dim AP with (m-rs)*G contiguous inner
# elements). Requires u negated (CCE supports add, not subtract): RA is
# negated in place after each recip, every DVE block update becomes add, and
# the final combine compensates the sign of q.
# Measured on HW: +100us/core at 0.35/14 (sim predicted -29us; the cost model
# underestimates SWDGE accum latency -- same verdict as the older wholesale
# experiment). Keep 0.
DMA_TAIL_FRAC = 0.0
DMA_MIN_M = 14   # only offload steps with m >= this (late steps lack slack)
LOAD_CHUNKS = 4  # split each A load into row-chunks (pipelines fill w/ compute)


_BOUNDS_CACHE: dict = {}


def _opt_bounds(mr: int, m: int):
    """Optimal row-block boundaries covering rows [0, mr) of the upper
    trapezoid, where a block [r, e) costs (e-r)*(m-r) streamed elements plus
    SPLIT_LAMBDA element-equivalents of per-op-pair overhead. Exact DP."""
    key = (mr, m, SPLIT_LAMBDA)
    if key in _BOUNDS_CACHE:
        return _BOUNDS_CACHE[key]
    lam = SPLIT_LAMBDA
    INF = float("inf")
    dp = [INF] * (mr + 1)
    nxt = [0] * (mr + 1)
    dp[mr] = 0.0
    for r in range(mr - 1, -1, -1):
        for e in range(r + 1, mr + 1):
            c = (e - r) * (m - r) + lam + dp[e]
            if c < dp[r]:
                dp[r], nxt[r] = c, e
    bounds = [0]
    r = 0
    while r < mr:
        r = nxt[r]
        bounds.append(r)
    _BOUNDS_CACHE[key] = bounds
    return bounds


def _strip_same_engine_dve_waits(nc):
    """Drop DVE-semaphore waits from instructions executing on the DVE.

    The DVE executes its queue strictly in order and flushes its pipeline
    (DRAIN) after every op before the next can issue, so RAW/WAR between two
    DVE instructions is enforced by hardware; the semaphore wait Tile emits
    for them only adds issue latency. Cross-engine waits are preserved.
    """
    n = 0
    for blk in nc.m.functions[0].blocks:
        for inst in blk.instructions:
            si = inst.sync_info
            if si is None or not si.on_wait:
                continue
            if str(getattr(inst, "engine", "")) != "EngineType.DVE":
                continue
            kept = [w for w in si.on_wait
                    if not (w.ant_name or "").startswith("DVE")]
            if len(kept) != len(si.on_wait):
                n += len(si.on_wait) - len(kept)
                inst.sync_info = mybir.SyncInfo(on_wait=kept,
                                                on_update=list(si.on_update))
    return n


def _emit_tile(nc, pools, yp, yt, vout, t):
    """Emit the full processing of one 128-problem tile."""
    apool, ppool, spool, _gpool, _upool = pools
    lo = t * 128
    A = apool.tile([128, CP1, CP1], F32, tag="A")
    D = spool.tile([128, C], F32, tag="D")
    # Sigma rows + mu land directly in M[0:40, :]: y_pred row-major
    # [40,41] matches M's first 40 rows, mu in column 40.
    nc.sync.dma_start(
        out=A[:, 0:C, :],
        in_=yp[lo : lo + 128, :].rearrange("p (i k) -> p i k", i=C),
    )
    nc.sync.dma_start(out=D[:], in_=yt[lo : lo + 128, :])
    # d = y_true - mu
    nc.vector.tensor_tensor(out=D[:], in0=D[:], in1=A[:, 0:C, C], op=OP.subtract)
    nc.vector.tensor_copy(out=A[:, 0:C, C], in_=D[:])   # border column
    nc.vector.tensor_copy(out=A[:, C, 0:C], in_=D[:])   # border row
    nc.vector.memset(A[:, C, C : C + 1], 0.0)           # corner

    for j in range(C):
        m = CP1 - 1 - j  # trailing block size
        R = spool.tile([128, 1], F32, tag="R")
        nc.vector.reciprocal(out=R[:], in_=A[:, j, j : j + 1])
        row = A[:, j, j + 1 : CP1]                      # [128, m] pivot row
        v_i = row[:, :, None].broadcast_to([128, m, m])  # w[i] over (i,k)
        v_k = row[:, None, :].broadcast_to([128, m, m])  # w[k] over (i,k)
        Pt = ppool.tile([128, m, m], F32, tag="P")
        nc.vector.scalar_tensor_tensor(
            out=Pt[:], in0=v_i, scalar=R[:], in1=v_k, op0=OP.mult, op1=OP.mult
        )
        nc.vector.tensor_tensor(
            out=A[:, j + 1 :, j + 1 :], in0=A[:, j + 1 :, j + 1 :], in1=Pt[:],
            op=OP.subtract,
        )

    # v = sum_j log(pivot_j) - corner   (corner = -d^T Sigma^-1 d)
    a = A[:]
    diag = bass.AP(tensor=a.tensor, offset=a.offset, ap=[a.ap[0], [CP1 + 1, C]])
    LOGT = spool.tile([128, C], F32, tag="LOG")
    S = spool.tile([128, 1], F32, tag="S")
    nc.scalar.activation(
        out=LOGT[:], in_=diag, func=mybir.ActivationFunctionType.Ln,
        accum_out=S[:],
    )
    V = spool.tile([128, 1], F32, tag="V")
    nc.vector.tensor_tensor(out=V[:], in0=S[:], in1=A[:, C, C : C + 1], op=OP.subtract)
    nc.sync.dma_start(out=vout[lo : lo + 128, :], in_=V[:])


def _make_pools(tc, ctx, G: int = 1):
    per_buf = G * C * CP1 * (2 if DT_A is F16 else 4)
    if DT_A is F16 and CAST_VIA_ACT:
        per_buf += G * C * CP1 * 4  # f32 staging tile shares the pool buf
    if per_buf <= 30 * 1024:
        abufs = ABUFS
    else:
        abufs = 2
    sbufs = SBUFS if G <= 4 else 3
    apool = ctx.enter_context(tc.tile_pool(name="A", bufs=abufs))
    ppool = ctx.enter_context(tc.tile_pool(name="P", bufs=PBUFS))
    gpool = ctx.enter_context(tc.tile_pool(name="gpP", bufs=PBUFS))
    spool = ctx.enter_context(tc.tile_pool(name="small", bufs=sbufs))
    upool = ctx.enter_context(tc.tile_pool(name="UM", bufs=UMBUFS))
    return apool, ppool, spool, gpool, upool


def _gp_split(bounds, m: int):
    """First block index of the suffix routed to GPSIMD: targets GP_FRAC of
    this step's streamed elements; block 0 always stays on DVE (it carries
    the next step's pivot row)."""
    els = [
        (bounds[b + 1] - bounds[b]) * (m - bounds[b])
        for b in range(len(bounds) - 1)
    ]
    budget = GP_FRAC * sum(els)
    acc, start = 0.0, len(els)
    for b in range(len(els) - 1, 0, -1):
        if abs(acc + els[b] - budget) >= abs(acc - budget):
            break
        acc += els[b]
        start = b
    return start


def _emit_tile_packed(nc, pools, yp, yt, vout, t, G, big_eng=None):
    """One tile = G*128 problems: G matrices packed along the free dim of
    each partition. Outputs one partially-summed value per partition.

    The border ROW (d^T) is never materialized: at step j the update only
    writes rows j+1..39 x cols j+1..40. Row j is final after step j, so the
    end-state matrix holds every pivot row; with the saved reciprocals the
    quadratic form is q = sum_j A[j,40]^2 / p_j.
    """
    apool, ppool, spool, gpool, upool = pools
    big = big_eng if big_eng is not None else nc.vector
    dt = DT_A
    lo = t * G * 128
    A = apool.tile([128, G, C, CP1], dt, tag="A")           # rows 0..39 only
    yp_ap = yp[lo : lo + G * 128, :].rearrange("(g p) (i k) -> p g i k", g=G, i=C)
    if dt is F32:
        nc.sync.dma_start(out=A[:], in_=yp_ap)
    elif CAST_VIA_ACT:
        AS = apool.tile([128, G, C, CP1], F32, tag="AS")
        nc.sync.dma_start(out=AS[:], in_=yp_ap)
        nc.scalar.activation(
            out=A[:], in_=AS[:], func=mybir.ActivationFunctionType.Copy)
    else:
        nc.gpsimd.dma_start(out=A[:], in_=yp_ap)           # SWDGE f32->f16 cast
    D = spool.tile([128, G, C], dt, tag="D")
    RA = spool.tile([128, G, C], dt, tag="RA")              # 1/pivot per step
    yt_ap = yt[lo : lo + G * 128, :].rearrange("(g p) c -> p g c", g=G)
    if dt is F32:
        nc.sync.dma_start(out=D[:], in_=yt_ap)
    else:
        nc.gpsimd.dma_start(out=D[:], in_=yt_ap)            # SWDGE f32->f16 cast
    # border column: d = y_true - mu  (mu is already in column 40)
    nc.vector.tensor_tensor(out=D[:], in0=D[:], in1=A[:, :, 0:C, C], op=OP.subtract)
    nc.vector.tensor_copy(out=A[:, :, 0:C, C], in_=D[:])

    U0 = spool.tile([128, G, C], dt, tag="U0")
    U1 = spool.tile([128, G, C], dt, tag="U1")
    Us = [U0, U1]

    def emit_recip_u(j):
        """recip_j, then u_j = row_j / p_j (into Us[j%2]); u skipped on the
        last step (no trailing rows)."""
        with nc.allow_low_precision(reason="per-pivot reciprocal, not an accum"):
            nc.vector.reciprocal(out=RA[:, :, j : j + 1], in_=A[:, :, j, j : j + 1])
        if j == C - 1:
            return
        m = C - j
        rj = A[:, :, j, j + 1 : CP1]
        if U_ON_ACT:
            for g in range(G):
                nc.scalar.activation(
                    out=Us[j % 2][:, g, 0:m], in_=A[:, g, j, j + 1 : CP1],
                    func=mybir.ActivationFunctionType.Copy,
                    scale=RA[:, g, j : j + 1],
                )
        else:
            ueng = nc.gpsimd if U_ON_GPSIMD else big
            ueng.tensor_tensor(
                out=Us[j % 2][:, :, 0:m], in0=rj,
                in1=RA[:, :, j : j + 1].broadcast_to([128, G, m]), op=OP.mult,
            )

    emit_recip_u(0)
    for j in range(C - 1):
        m = C - j  # trailing columns j+1..40 (incl. border col) = m
        U = Us[j % 2]
        row = A[:, :, j, j + 1 : CP1]                       # [128, G, m]
        mr = m - 1                                          # rows j+1..39
        # Only entries (i, k>=i) plus the border column are ever read later;
        # cover the upper trapezoid with row-blocks whose columns start at
        # the block's first row (bounding rectangles).
        if USE_DP_BOUNDS:
            bounds = _opt_bounds(mr, m)
        else:
            nb = 1 + sum(mr >= th for th in SPLIT_THRESHOLDS)
            bounds = [rs * mr // nb for rs in range(nb)] + [mr]
        gp_start = (
            _gp_split(bounds, m)
            if (GP_FRAC > 0 and big_eng is None and not SUB_VIA_DMA)
            else len(bounds)
        )
        for b in range(len(bounds) - 1):
            rs, re = bounds[b], bounds[b + 1]
            nrows = re - rs
            v_i = U[:, :, rs:re, None].broadcast_to([128, G, nrows, m - rs])
            v_k = row[:, :, None, rs:m].broadcast_to([128, G, nrows, m - rs])
            if SUB_VIA_DMA:
                # Pt holds -(w/p) (x) w in a full-row-width (41) padded tile
                # so the accumulate DMA collapses to a 3-dim AP. Columns left
                # of the block's start hold stale garbage that lands in
                # strictly-lower cells of A, which are never read.
                Pt = ppool.tile([128, G, nrows, CP1], dt, tag="P")
                big.tensor_tensor(
                    out=Pt[:, :, :, j + 1 + rs : CP1], in0=v_i, in1=v_k, op=OP.mult
                )
                blk_full = A[:, :, j + 1 + rs : j + 1 + re, :]
                nc.gpsimd.dma_start(out=blk_full, in_=Pt[:], accum_op=OP.add)
            else:
                on_gp = b >= gp_start
                eng = nc.gpsimd if on_gp else big
                pool = gpool if (on_gp or big_eng is not None) else ppool
                Pt = pool.tile([128, G, nrows, m - rs], dt, tag="P")
                if USE_ACT_UM and not on_gp:
                    UM = upool.tile([128, G, nrows, m - rs], dt, tag="UM")
                    nc.scalar.activation(
                        out=UM[:], in_=v_i,
                        func=mybir.ActivationFunctionType.Copy,
                    )
                    eng.tensor_tensor(out=Pt[:], in0=UM[:], in1=v_k, op=OP.mult)
                else:
                    eng.tensor_tensor(out=Pt[:], in0=v_i, in1=v_k, op=OP.mult)
                blk = A[:, :, j + 1 + rs : j + 1 + re, j + 1 + rs :]
                eng.tensor_tensor(out=blk, in0=blk, in1=Pt[:], op=OP.subtract)
            if b == 0:
                # row j+1 is final: issue the next step's recip + u now so
                # the cross-step chain doesn't wait on this step's tail.
                emit_recip_u(j + 1)

    # per-partition partial sum over g: sum_j log(p_j) + sum_j dcol_j^2/p_j
    a = A[:]
    diag = bass.AP(
        tensor=a.tensor, offset=a.offset,
        ap=[a.ap[0], [C * CP1, G], [CP1 + 1, C]],
    )
    LOGT = spool.tile([128, G, C], dt, tag="LOG")
    S = spool.tile([128, 1], F32, tag="S")
    nc.scalar.activation(
        out=LOGT[:], in_=diag, func=mybir.ActivationFunctionType.Ln,
        accum_out=S[:],
    )
    dcol = A[:, :, 0:C, C]                                  # final border col
    SQ = spool.tile([128, G, C], dt, tag="SQ")
    nc.vector.tensor_tensor(out=SQ[:], in0=dcol, in1=dcol, op=OP.mult)
    Q = spool.tile([128, 1], F32, tag="Q")
    nc.vector.scalar_tensor_tensor(
        out=SQ[:], in0=SQ[:], scalar=1.0, in1=RA[:], op0=OP.mult, op1=OP.mult,
        accum_out=Q[:],
    )
    V = spool.tile([128, 1], F32, tag="V")
    nc.vector.tensor_tensor(out=V[:], in0=S[:], in1=Q[:], op=OP.add)
    nc.sync.dma_start(out=vout[t * 128 : (t + 1) * 128, :], in_=V[:])


def _emit_tile_ginner(nc, pools, yp, vout, t, G):
    """g-innermost variant: one tile = G*128 problems stored interleaved as
    A[128, C, CP1, G] (fp16), i.e. element (i,k) of all G matrices adjacent.

    Every update operand then has innermost stride 1 over g (16 contiguous
    fp16 elements), so the outer-product mult, the subtract, AND the u scale
    all qualify for the DVE 2x_1p packed mode without materializing any
    broadcast operand -- the Scalar engine is not needed at all. The host
    pre-transposes y_pred/y_true to this layout so the load DMA stays dense.
    """
    apool, ppool, spool, _gpool, _upool = pools
    big = nc.vector
    dt = DT_A
    lo = t * 128
    A = apool.tile([128, C, CP1, G], dt, tag="A")
    # border column already holds d = y_true - mu (folded on the host).
    # Load in row-chunks so step-0 blocks start as soon as their rows land
    # (cuts the initial fill gap ~22us -> ~6us; Tile's data deps do the rest).
    rows_per = (C + LOAD_CHUNKS - 1) // LOAD_CHUNKS
    for r0 in range(0, C, rows_per):
        r1 = min(C, r0 + rows_per)
        nc.gpsimd.dma_start(
            out=A[:, r0:r1, :, :],
            in_=yp[lo : lo + 128, r0 * CP1 * G : r1 * CP1 * G].rearrange(
                "p (i k g) -> p i k g", i=r1 - r0, k=CP1))  # SWDGE f32->f16 cast
    RA = spool.tile([128, C, G], dt, tag="RA")            # 1/pivot per step

    U0 = spool.tile([128, C, G], dt, tag="U0")
    U1 = spool.tile([128, C, G], dt, tag="U1")
    Us = [U0, U1]

    neg_u = DMA_TAIL_FRAC > 0
    upd_op = OP.add if neg_u else OP.subtract

    def emit_recip_u(j):
        with nc.allow_low_precision(reason="per-pivot reciprocal, not an accum"):
            nc.vector.reciprocal(out=RA[:, j, :], in_=A[:, j, j, :])
        if neg_u:
            # RA holds -1/p: u comes out negated (updates become adds, the
            # accum-DMA's CCE only has add) and q's sign is fixed at the end.
            nc.vector.tensor_scalar(
                out=RA[:, j, :], in0=RA[:, j, :], scalar1=-1.0, scalar2=None,
                op0=OP.mult)
        if j == C - 1:
            return
        m = C - j
        big.tensor_tensor(
            out=Us[j % 2][:, 0:m, :], in0=A[:, j, j + 1 : CP1, :],
            in1=RA[:, j, None, :].broadcast_to([128, m, G]), op=OP.mult,
        )

    emit_recip_u(0)
    for j in range(C - 1):
        m = C - j
        U = Us[j % 2]
        row = A[:, j, j + 1 : CP1, :]                     # [128, m, G]
        mr = m - 1
        ts = mr  # first row of the DMA-offloaded tail block
        if DMA_TAIL_FRAC > 0 and m >= DMA_MIN_M:
            ts = int(mr * (1.0 - DMA_TAIL_FRAC))
            if mr - ts < 4:
                ts = mr
        if USE_DP_BOUNDS:
            bounds = _opt_bounds(ts, m)
        else:
            nb = 1 + sum(ts >= th for th in SPLIT_THRESHOLDS)
            bounds = [rs * ts // nb for rs in range(nb)] + [ts]
        if ts < mr:
            bounds = bounds + [mr]
        for b in range(len(bounds) - 1):
            rs, re = bounds[b], bounds[b + 1]
            nrows = re - rs
            v_i = U[:, rs:re, None, :].broadcast_to([128, nrows, m - rs, G])
            v_k = row[:, None, rs:m, :].broadcast_to([128, nrows, m - rs, G])
            Pt = ppool.tile([128, nrows, m - rs, G], dt, tag="P")
            big.tensor_tensor(out=Pt[:], in0=v_i, in1=v_k, op=OP.mult)
            blk = A[:, j + 1 + rs : j + 1 + re, j + 1 + rs :, :]
            if rs >= ts:
                nc.gpsimd.dma_start(out=blk, in_=Pt[:], accum_op=OP.add)
            else:
                big.tensor_tensor(out=blk, in0=blk, in1=Pt[:], op=upd_op)
            if b == 0:
                emit_recip_u(j + 1)

    # per-partition partial sum: sum_j log(p_j) + sum_j dcol_j^2 / p_j
    a = A[:]
    diag = bass.AP(
        tensor=a.tensor, offset=a.offset,
        ap=[a.ap[0], [(CP1 + 1) * G, C], [1, G]],
    )
    LOGT = spool.tile([128, C, G], dt, tag="LOG")
    S = spool.tile([128, 1], F32, tag="S")
    nc.scalar.activation(
        out=LOGT[:], in_=diag, func=mybir.ActivationFunctionType.Ln,
        accum_out=S[:],
    )
    dcol = A[:, 0:C, C, :]                                # final border col
    SQ = spool.tile([128, C, G], dt, tag="SQ")
    nc.vector.tensor_tensor(out=SQ[:], in0=dcol, in1=dcol, op=OP.mult)
    Q = spool.tile([128, 1], F32, tag="Q")
    nc.vector.scalar_tensor_tensor(
        out=SQ[:], in0=SQ[:], scalar=1.0, in1=RA[:], op0=OP.mult, op1=OP.mult,
        accum_out=Q[:],
    )
    V = spool.tile([128, 1], F32, tag="V")
    # with u negated, RA holds -1/p so Q = -q: compensate here
    nc.vector.tensor_tensor(
        out=V[:], in0=S[:], in1=Q[:], op=OP.subtract if neg_u else OP.add)
    nc.sync.dma_start(out=vout[t * 128 : (t + 1) * 128, :], in_=V[:])


def build3(ntiles: int, G: int) -> bass.Bass:
    """g-innermost packed variant (host pre-transposed inputs)."""
    nc = bacc.Bacc("TRN2", target_bir_lowering=False)
    yp = nc.dram_tensor("y_pred", [ntiles * 128, C * CP1 * G], F32,
                        kind="ExternalInput")
    vout = nc.dram_tensor("v_out", [ntiles * 128, 1], F32, kind="ExternalOutput")
    with TileContext(nc) as tc, ExitStack() as ctx:
        pools = _make_pools(tc, ctx, G)
        for t in range(ntiles):
            _emit_tile_ginner(nc, pools, yp, vout, t, G)
    if not nc.is_finalized():
        nc.finalize()
    return nc


def build_loop3(body_tiles: int, reps: int, G: int) -> bass.Bass:
    nc = bacc.Bacc("TRN2", target_bir_lowering=False)
    yp = nc.dram_tensor("y_pred", [body_tiles * 128, C * CP1 * G], F32,
                        kind="ExternalInput")
    vout = nc.dram_tensor("v_out", [body_tiles * 128, 1], F32,
                          kind="ExternalOutput")
    with TileContext(nc) as tc, ExitStack() as ctx:
        pools = _make_pools(tc, ctx, G)

        def body(i, unroll=1):
            for t in range(body_tiles):
                _emit_tile_ginner(nc, pools, yp, vout, t, G)

        with tc.For_i(0, reps, 1) as i:
            body(i)
    if not nc.is_finalized():
        nc.finalize()
    return nc


BORDER_IDX = np.arange(C) * CP1 + C


def to_ginner(ypf: np.ndarray, ytf: np.ndarray, ntiles: int, G: int):
    """Host-side relayout: per tile, problems (g, p) -> partition p holds the
    G matrices interleaved element-wise: row p = y_pred[(g,p), i, k] laid out
    as (i, k, g). The border column is replaced by d = y_true - mu so the
    device needs neither y_true nor the border subtract."""
    n = ntiles * G * 128
    yp = ypf[:n].copy()
    yp[:, BORDER_IDX] = ytf[:n] - yp[:, BORDER_IDX]
    yp = yp.reshape(ntiles, G, 128, C * CP1)
    yp = yp.transpose(0, 2, 3, 1).reshape(ntiles * 128, C * CP1 * G)
    return np.ascontiguousarray(yp)


def build(ntiles: int = NTILES_FULL) -> bass.Bass:
    nprob = ntiles * 128
    nc = bacc.Bacc("TRN2", target_bir_lowering=False)
    yp = nc.dram_tensor("y_pred", [nprob, C * CP1], F32, kind="ExternalInput")
    yt = nc.dram_tensor("y_true", [nprob, C], F32, kind="ExternalInput")
    vout = nc.dram_tensor("v_out", [nprob, 1], F32, kind="ExternalOutput")

    with TileContext(nc) as tc, ExitStack() as ctx:
        pools = _make_pools(tc, ctx)
        for t in range(ntiles):
            _emit_tile(nc, pools, yp, yt, vout, t)
    if not nc.is_finalized():
        nc.finalize()
    return nc


def build_loop(body_tiles: int, reps: int) -> bass.Bass:
    """Timing amplifier: process the same `body_tiles` tiles `reps` times
    inside a For_i loop (static addressing; WAW across reps is fine)."""
    nprob = body_tiles * 128
    nc = bacc.Bacc("TRN2", target_bir_lowering=False)
    yp = nc.dram_tensor("y_pred", [nprob, C * CP1], F32, kind="ExternalInput")
    yt = nc.dram_tensor("y_true", [nprob, C], F32, kind="ExternalInput")
    vout = nc.dram_tensor("v_out", [nprob, 1], F32, kind="ExternalOutput")

    with TileContext(nc) as tc, ExitStack() as ctx:
        pools = _make_pools(tc, ctx)

        def body(i, unroll=1):
            for t in range(body_tiles):
                _emit_tile(nc, pools, yp, yt, vout, t)

        with tc.For_i(0, reps, 1) as i:
            body(i)
    if not nc.is_finalized():
        nc.finalize()
    return nc


def build2(ntiles: int, G: int) -> bass.Bass:
    """Packed variant: each tile covers G*128 problems."""
    nprob = ntiles * G * 128
    nc = bacc.Bacc("TRN2", target_bir_lowering=False)
    yp = nc.dram_tensor("y_pred", [nprob, C * CP1], F32, kind="ExternalInput")
    yt = nc.dram_tensor("y_true", [nprob, C], F32, kind="ExternalInput")
    vout = nc.dram_tensor("v_out", [ntiles * 128, 1], F32, kind="ExternalOutput")

    with TileContext(nc) as tc, ExitStack() as ctx:
        pools = _make_pools(tc, ctx, G)
        for t in range(ntiles):
            _emit_tile_packed(nc, pools, yp, yt, vout, t, G)
    if STRIP_DVE_WAITS:
        _strip_same_engine_dve_waits(nc)
    if not nc.is_finalized():
        nc.finalize()
    return nc


def build_loop2(body_tiles: int, reps: int, G: int, gp_every: int = 0,
                gp_tiles=()) -> bass.Bass:
    """gp_every=k: every k-th tile runs its big ops on GPSIMD (0 = never).
    gp_tiles: explicit tile indices to run on GPSIMD (overrides gp_every)."""
    nprob = body_tiles * G * 128
    nc = bacc.Bacc("TRN2", target_bir_lowering=False)
    yp = nc.dram_tensor("y_pred", [nprob, C * CP1], F32, kind="ExternalInput")
    yt = nc.dram_tensor("y_true", [nprob, C], F32, kind="ExternalInput")
    vout = nc.dram_tensor("v_out", [body_tiles * 128, 1], F32, kind="ExternalOutput")

    with TileContext(nc) as tc, ExitStack() as ctx:
        pools = _make_pools(tc, ctx, G)

        def body(i, unroll=1):
            for t in range(body_tiles):
                on_gp = (t in gp_tiles) or (gp_every and t % gp_every == gp_every - 1)
                eng = nc.gpsimd if on_gp else None
                _emit_tile_packed(nc, pools, yp, yt, vout, t, G, big_eng=eng)

        with tc.For_i(0, reps, 1) as i:
            body(i)
    if STRIP_DVE_WAITS:
        _strip_same_engine_dve_waits(nc)
    if not nc.is_finalized():
        nc.finalize()
    return nc


_CACHE: dict = {}


def _pad_rows(n_pad: int) -> tuple[np.ndarray, np.ndarray]:
    """Identity problems: Sigma=I, mu=0, y_true=0 -> v contribution exactly 0."""
    row = np.concatenate([np.eye(C, dtype=np.float32), np.zeros((C, 1), np.float32)], axis=1)
    return (
        np.tile(row.reshape(1, -1), (n_pad, 1)),
        np.zeros((n_pad, C), np.float32),
    )


G_PACK = 16
NTILES_PACKED = PAD // (G_PACK * 128)  # 2


def kernel(y_true: np.ndarray, y_pred: np.ndarray) -> np.ndarray:
    # np.asarray also handles jax arrays (device -> host copy)
    ypf = np.ascontiguousarray(
        np.asarray(y_pred, dtype=np.float32).reshape(NPROB, C * CP1))
    ytf = np.ascontiguousarray(
        np.asarray(y_true, dtype=np.float32).reshape(NPROB, C))

    if "nc" not in _CACHE:
        _CACHE["nc"] = build3(NTILES_PACKED, G_PACK)
    nc = _CACHE["nc"]

    pad_p, pad_t = _pad_rows(PAD - PER_CORE)
    in_maps = []
    for c in range(N_CORES):
        sl = slice(c * PER_CORE, (c + 1) * PER_CORE)
        ypg = to_ginner(
            np.concatenate([ypf[sl], pad_p], axis=0),
            np.concatenate([ytf[sl], pad_t], axis=0),
            NTILES_PACKED, G_PACK,
        )
        in_maps.append({"y_pred": ypg})

    # Transient device flakes (observed ~once per dozen runs) can yield NaN;
    # the result is cheap to validate, so retry a couple of times on
    # non-finite output before giving up.
    for _attempt in range(3):
        res = run_bass_kernel_spmd(nc, in_maps, core_ids=list(range(N_CORES)))
        # v_out rows are per-partition partial sums (padding contributes 0)
        v = np.concatenate([r["v_out"][:, 0] for r in res.results])
        loss = 0.5 * float(np.sum(v, dtype=np.float64)) / B + T * 0.5 * C * LOG_2PI
        if np.isfinite(loss):
            break
    return np.float32(loss)



# revision 39
# speedup vs baseline: 1.2564x; 1.0581x over previous
"""Trainium2 Bass kernel for AdversarialLogLikelihoodLossLayer.

Per (b,t): negative log-likelihood of a C=40-dim Gaussian
    nll = 0.5*(d^T Sigma^-1 d + logdet Sigma + C*log(2pi)),  d = y_true - mu
summed over T, meaned over B -> scalar.

Algorithm: batched bordered LDL^T (no sqrt, no pivoting; Sigma is SPD and
well-conditioned). Per problem form M = [Sigma | d] (40x41; the d border
column replaces mu in-place after one subtract; the border row is never
materialized). 40 rank-1 Schur eliminations give pivots p_j with
logdet = sum_j log p_j, and since row j is final after step j, the end-state
border column holds w_j[40], so q = d^T Sigma^-1 d = sum_j M[j,40]^2 / p_j
using the saved pivot reciprocals.

Layout (the key trick): 128 problems across SBUF partitions x G=16 matrices
packed G-INNERMOST -- A[128, 40, 41, G] fp16, i.e. element (i,k) of all 16
matrices adjacent. Every DVE operand (the two stride-0 broadcast operands of
the outer product, the subtract, and the u = w/p scale) then has innermost
stride 1 over g with 16 contiguous fp16 elements, which qualifies ALL of
them for the DVE 2x_1p packed mode (the RTL condition is on the innermost
dim only: 2-byte dtype, step +-1, >=2 elements). In the older g-major layout
the broadcast operand had innermost stride 0, pinning the product pass to
1x; a prior iteration fixed that by materializing the broadcast densely on
the Scalar engine (803us -> 637us) before this layout made the ACT bridge
unnecessary (637us -> 567us, removing its cross-engine handshake overhead
too -- HW-vs-costmodel gap shrank from ~31us to ~10us/tile). The host
pre-transposes y_pred into the interleaved layout (to_ginner) so the SWDGE
cast DMA stays dense on both sides, and folds d = y_true - mu into the
border column there (y_true never reaches the device; removes one DMA
stream and the head-of-tile border ops). fp16 inputs are cast on-device by
the SWDGE DMA (loss rel err ~3e-5). Each A load is split into LOAD_CHUNKS=4
row-chunk DMAs so step-0 blocks start as soon as their rows land and
rep-boundary WAR deps resolve per chunk (571 -> ~550-558us measured; 8
chunks regressed to 662us -- real SWDGE per-DMA cost, which CoreSim
underestimates, overtakes the pipelining gain).

Each step's update runs as DVE tensor_tensor pairs covering only the upper
trapezoid + border column -- the strictly-lower triangle is never read by
later steps, so it is skipped via row-blocks whose columns start at the
block's first row. Block bounds come from an exact DP with per-block
overhead lambda=10 element-equivalents (~2 ops x ~50ns marginal, measured
via CoreSim which tracks HW deltas well). The next step's reciprocal and u
are issued right after the first row-block of the current step (row j+1 is
final then), with double-buffered u. Pivot logs are summed in one ScalarE
Ln+accum op. CoreSim (no_exec) puts DVE occupancy at ~91% with the stream
at the 2x floor (~204us/tile) + ~34us/tile op overhead; remaining levers
are small.

Measured dead ends (do not revisit without new evidence): GPSIMD offload of
block pairs or whole tiles (GPSIMD's SBUF port is physically shared with
the DVE -- the "POOL slot" -- zero overlap, mixed schedules run at the
serial sum); SWDGE accumulate-DMA for the subtract (per-DMA latency on the
40-step chain, 3x slower); u scaled via an ACT-materialized reciprocal
(two extra cross-engine hops on the per-step critical chain, +53us);
stripping same-engine DVE waits (silently corrupts results); coarse DP
bounds when ACT was in the loop (slack elements then cost ACT time too);
DMA_TAIL_FRAC accum-DMA offload of tail-block subtracts (+100us/core on HW
at 0.35 despite CoreSim predicting -29us -- the cost model underestimates
SWDGE accum latency; trust CoreSim for DVE scheduling deltas, NOT for
SWDGE costs).
Considered and rejected: custom fused DVE ops (always 1x -- now strictly
worse than the 2x stock-op pair); TensorE (cannot batch per-partition
independent tiny matmuls); log/Neumann series for logdet/solve (needs
per-problem matmuls anyway).

Data parallel over 8 NeuronCores: 32000 problems -> 4000/core, padded to
4096 = 2 packed tiles with identity problems (which contribute exactly 0);
per-partition partial sums are reduced on the host in float64.
"""

import sys
from contextlib import ExitStack

import numpy as np

sys.path.insert(0, "/opt/trn_rl_repo")

import concourse.bacc as bacc  # noqa: E402
import concourse.bass as bass  # noqa: E402
from concourse import mybir  # noqa: E402
from concourse.bass_utils import run_bass_kernel_spmd  # noqa: E402
from concourse.tile import TileContext  # noqa: E402

B, T, C = 64, 500, 40
CP1 = C + 1
N_CORES = 8
NPROB = B * T
PER_CORE = NPROB // N_CORES  # 4000
PAD = 4096                   # per-core padded problem count
NTILES_FULL = PAD // 128     # 32

F32 = mybir.dt.float32
F16 = mybir.dt.float16
OP = mybir.AluOpType
DT_A = F16      # dtype of the working matrix / products (F16 -> 2x subtract)
CAST_VIA_ACT = False  # False: SWDGE cast DMA; True: HWDGE + ACT copy-cast
LOG_2PI = float(np.log(2.0 * np.pi))
SPLIT_THRESHOLDS = (3, 5, 8, 11, 15, 19, 23, 27, 31, 35, 39)
SPLIT_LAMBDA = 10.0
USE_DP_BOUNDS = True
ABUFS = 4
PBUFS = 2
SBUFS = 6
U_ON_GPSIMD = False
U_ON_ACT = False
STRIP_DVE_WAITS = False  # UNSAFE: silently corrupts results (races); keep off
SUB_VIA_DMA = False
GP_FRAC = 0.0   # fraction of block elements routed to GPSIMD (0 = all DVE)
# Legacy g-major path only (_emit_tile_packed): materialize the broadcast
# u-operand densely on the Scalar engine so the DVE product gets 2x_1p.
# The g-innermost layout (_emit_tile_ginner, used by kernel()) makes every
# operand 2x-eligible directly and does not need ACT at all.
USE_ACT_UM = True
UMBUFS = 4
# g-inner path: offload the subtract of the bottom DMA_TAIL_FRAC rows of each
# step's update to a SWDGE accumulate-DMA (Pool engine + DMA are ~idle; in the
# g-inner layout the tail block is one 3-dim AP with (m-rs)*G contiguous inner
# elements). Requires u negated (CCE supports add, not subtract): RA is
# negated in place after each recip, every DVE block update becomes add, and
# the final combine compensates the sign of q.
# Measured on HW: +100us/core at 0.35/14 (sim predicted -29us; the cost model
# underestimates SWDGE accum latency -- same verdict as the older wholesale
# experiment). Keep 0.
DMA_TAIL_FRAC = 0.0
DMA_MIN_M = 14   # only offload steps with m >= this (late steps lack slack)
LOAD_CHUNKS = 4  # split each A load into row-chunks (pipelines fill w/ compute)


_BOUNDS_CACHE: dict = {}


def _opt_bounds(mr: int, m: int):
    """Optimal row-block boundaries covering rows [0, mr) of the upper
    trapezoid, where a block [r, e) costs (e-r)*(m-r) streamed elements plus
    SPLIT_LAMBDA element-equivalents of per-op-pair overhead. Exact DP."""
    key = (mr, m, SPLIT_LAMBDA)
    if key in _BOUNDS_CACHE:
        return _BOUNDS_CACHE[key]
    lam = SPLIT_LAMBDA
    INF = float("inf")
    dp = [INF] * (mr + 1)
    nxt = [0] * (mr + 1)
    dp[mr] = 0.0
    for r in range(mr - 1, -1, -1):
        for e in range(r + 1, mr + 1):
            c = (e - r) * (m - r) + lam + dp[e]
            if c < dp[r]:
                dp[r], nxt[r] = c, e
    bounds = [0]
    r = 0
    while r < mr:
        r = nxt[r]
        bounds.append(r)
    _BOUNDS_CACHE[key] = bounds
    return bounds


def _strip_same_engine_dve_waits(nc):
    """Drop DVE-semaphore waits from instructions executing on the DVE.

    The DVE executes its queue strictly in order and flushes its pipeline
    (DRAIN) after every op before the next can issue, so RAW/WAR between two
    DVE instructions is enforced by hardware; the semaphore wait Tile emits
    for them only adds issue latency. Cross-engine waits are preserved.
    """
    n = 0
    for blk in nc.m.functions[0].blocks:
        for inst in blk.instructions:
            si = inst.sync_info
            if si is None or not si.on_wait:
                continue
            if str(getattr(inst, "engine", "")) != "EngineType.DVE":
                continue
            kept = [w for w in si.on_wait
                    if not (w.ant_name or "").startswith("DVE")]
            if len(kept) != len(si.on_wait):
                n += len(si.on_wait) - len(kept)
                inst.sync_info = mybir.SyncInfo(on_wait=kept,
                                                on_update=list(si.on_update))
    return n


def _emit_tile(nc, pools, yp, yt, vout, t):
    """Emit the full processing of one 128-problem tile."""
    apool, ppool, spool, _gpool, _upool = pools
    lo = t * 128
    A = apool.tile([128, CP1, CP1], F32, tag="A")
    D = spool.tile([128, C], F32, tag="D")
    # Sigma rows + mu land directly in M[0:40, :]: y_pred row-major
    # [40,41] matches M's first 40 rows, mu in column 40.
    nc.sync.dma_start(
        out=A[:, 0:C, :],
        in_=yp[lo : lo + 128, :].rearrange("p (i k) -> p i k", i=C),
    )
    nc.sync.dma_start(out=D[:], in_=yt[lo : lo + 128, :])
    # d = y_true - mu
    nc.vector.tensor_tensor(out=D[:], in0=D[:], in1=A[:, 0:C, C], op=OP.subtract)
    nc.vector.tensor_copy(out=A[:, 0:C, C], in_=D[:])   # border column
    nc.vector.tensor_copy(out=A[:, C, 0:C], in_=D[:])   # border row
    nc.vector.memset(A[:, C, C : C + 1], 0.0)           # corner

    for j in range(C):
        m = CP1 - 1 - j  # trailing block size
        R = spool.tile([128, 1], F32, tag="R")
        nc.vector.reciprocal(out=R[:], in_=A[:, j, j : j + 1])
        row = A[:, j, j + 1 : CP1]                      # [128, m] pivot row
        v_i = row[:, :, None].broadcast_to([128, m, m])  # w[i] over (i,k)
        v_k = row[:, None, :].broadcast_to([128, m, m])  # w[k] over (i,k)
        Pt = ppool.tile([128, m, m], F32, tag="P")
        nc.vector.scalar_tensor_tensor(
            out=Pt[:], in0=v_i, scalar=R[:], in1=v_k, op0=OP.mult, op1=OP.mult
        )
        nc.vector.tensor_tensor(
            out=A[:, j + 1 :, j + 1 :], in0=A[:, j + 1 :, j + 1 :], in1=Pt[:],
            op=OP.subtract,
        )

    # v = sum_j log(pivot_j) - corner   (corner = -d^T Sigma^-1 d)
    a = A[:]
    diag = bass.AP(tensor=a.tensor, offset=a.offset, ap=[a.ap[0], [CP1 + 1, C]])
    LOGT = spool.tile([128, C], F32, tag="LOG")
    S = spool.tile([128, 1], F32, tag="S")
    nc.scalar.activation(
        out=LOGT[:], in_=diag, func=mybir.ActivationFunctionType.Ln,
        accum_out=S[:],
    )
    V = spool.tile([128, 1], F32, tag="V")
    nc.vector.tensor_tensor(out=V[:], in0=S[:], in1=A[:, C, C : C + 1], op=OP.subtract)
    nc.sync.dma_start(out=vout[lo : lo + 128, :], in_=V[:])


def _make_pools(tc, ctx, G: int = 1):
    per_buf = G * C * CP1 * (2 if DT_A is F16 else 4)
    if DT_A is F16 and CAST_VIA_ACT:
        per_buf += G * C * CP1 * 4  # f32 staging tile shares the pool buf
    if per_buf <= 30 * 1024:
        abufs = ABUFS
    else:
        abufs = 2
    sbufs = SBUFS if G <= 4 else 3
    apool = ctx.enter_context(tc.tile_pool(name="A", bufs=abufs))
    ppool = ctx.enter_context(tc.tile_pool(name="P", bufs=PBUFS))
    gpool = ctx.enter_context(tc.tile_pool(name="gpP", bufs=PBUFS))
    spool = ctx.enter_context(tc.tile_pool(name="small", bufs=sbufs))
    upool = ctx.enter_context(tc.tile_pool(name="UM", bufs=UMBUFS))
    return apool, ppool, spool, gpool, upool


def _gp_split(bounds, m: int):
    """First block index of the suffix routed to GPSIMD: targets GP_FRAC of
    this step's streamed elements; block 0 always stays on DVE (it carries
    the next step's pivot row)."""
    els = [
        (bounds[b + 1] - bounds[b]) * (m - bounds[b])
        for b in range(len(bounds) - 1)
    ]
    budget = GP_FRAC * sum(els)
    acc, start = 0.0, len(els)
    for b in range(len(els) - 1, 0, -1):
        if abs(acc + els[b] - budget) >= abs(acc - budget):
            break
        acc += els[b]
        start = b
    return start


def _emit_tile_packed(nc, pools, yp, yt, vout, t, G, big_eng=None):
    """One tile = G*128 problems: G matrices packed along the free dim of
    each partition. Outputs one partially-summed value per partition.

    The border ROW (d^T) is never materialized: at step j the update only
    writes rows j+1..39 x cols j+1..40. Row j is final after step j, so the
    end-state matrix holds every pivot row; with the saved reciprocals the
    quadratic form is q = sum_j A[j,40]^2 / p_j.
    """
    apool, ppool, spool, gpool, upool = pools
    big = big_eng if big_eng is not None else nc.vector
    dt = DT_A
    lo = t * G * 128
    A = apool.tile([128, G, C, CP1], dt, tag="A")           # rows 0..39 only
    yp_ap = yp[lo : lo + G * 128, :].rearrange("(g p) (i k) -> p g i k", g=G, i=C)
    if dt is F32:
        nc.sync.dma_start(out=A[:], in_=yp_ap)
    elif CAST_VIA_ACT:
        AS = apool.tile([128, G, C, CP1], F32, tag="AS")
        nc.sync.dma_start(out=AS[:], in_=yp_ap)
        nc.scalar.activation(
            out=A[:], in_=AS[:], func=mybir.ActivationFunctionType.Copy)
    else:
        nc.gpsimd.dma_start(out=A[:], in_=yp_ap)           # SWDGE f32->f16 cast
    D = spool.tile([128, G, C], dt, tag="D")
    RA = spool.tile([128, G, C], dt, tag="RA")              # 1/pivot per step
    yt_ap = yt[lo : lo + G * 128, :].rearrange("(g p) c -> p g c", g=G)
    if dt is F32:
        nc.sync.dma_start(out=D[:], in_=yt_ap)
    else:
        nc.gpsimd.dma_start(out=D[:], in_=yt_ap)            # SWDGE f32->f16 cast
    # border column: d = y_true - mu  (mu is already in column 40)
    nc.vector.tensor_tensor(out=D[:], in0=D[:], in1=A[:, :, 0:C, C], op=OP.subtract)
    nc.vector.tensor_copy(out=A[:, :, 0:C, C], in_=D[:])

    U0 = spool.tile([128, G, C], dt, tag="U0")
    U1 = spool.tile([128, G, C], dt, tag="U1")
    Us = [U0, U1]

    def emit_recip_u(j):
        """recip_j, then u_j = row_j / p_j (into Us[j%2]); u skipped on the
        last step (no trailing rows)."""
        with nc.allow_low_precision(reason="per-pivot reciprocal, not an accum"):
            nc.vector.reciprocal(out=RA[:, :, j : j + 1], in_=A[:, :, j, j : j + 1])
        if j == C - 1:
            return
        m = C - j
        rj = A[:, :, j, j + 1 : CP1]
        if U_ON_ACT:
            for g in range(G):
                nc.scalar.activation(
                    out=Us[j % 2][:, g, 0:m], in_=A[:, g, j, j + 1 : CP1],
                    func=mybir.ActivationFunctionType.Copy,
                    scale=RA[:, g, j : j + 1],
                )
        else:
            ueng = nc.gpsimd if U_ON_GPSIMD else big
            ueng.tensor_tensor(
                out=Us[j % 2][:, :, 0:m], in0=rj,
                in1=RA[:, :, j : j + 1].broadcast_to([128, G, m]), op=OP.mult,
            )

    emit_recip_u(0)
    for j in range(C - 1):
        m = C - j  # trailing columns j+1..40 (incl. border col) = m
        U = Us[j % 2]
        row = A[:, :, j, j + 1 : CP1]                       # [128, G, m]
        mr = m - 1                                          # rows j+1..39
        # Only entries (i, k>=i) plus the border column are ever read later;
        # cover the upper trapezoid with row-blocks whose columns start at
        # the block's first row (bounding rectangles).
        if USE_DP_BOUNDS:
            bounds = _opt_bounds(mr, m)
        else:
            nb = 1 + sum(mr >= th for th in SPLIT_THRESHOLDS)
            bounds = [rs * mr // nb for rs in range(nb)] + [mr]
        gp_start = (
            _gp_split(bounds, m)
            if (GP_FRAC > 0 and big_eng is None and not SUB_VIA_DMA)
            else len(bounds)
        )
        for b in range(len(bounds) - 1):
            rs, re = bounds[b], bounds[b + 1]
            nrows = re - rs
            v_i = U[:, :, rs:re, None].broadcast_to([128, G, nrows, m - rs])
            v_k = row[:, :, None, rs:m].broadcast_to([128, G, nrows, m - rs])
            if SUB_VIA_DMA:
                # Pt holds -(w/p) (x) w in a full-row-width (41) padded tile
                # so the accumulate DMA collapses to a 3-dim AP. Columns left
                # of the block's start hold stale garbage that lands in
                # strictly-lower cells of A, which are never read.
                Pt = ppool.tile([128, G, nrows, CP1], dt, tag="P")
                big.tensor_tensor(
                    out=Pt[:, :, :, j + 1 + rs : CP1], in0=v_i, in1=v_k, op=OP.mult
                )
                blk_full = A[:, :, j + 1 + rs : j + 1 + re, :]
                nc.gpsimd.dma_start(out=blk_full, in_=Pt[:], accum_op=OP.add)
            else:
                on_gp = b >= gp_start
                eng = nc.gpsimd if on_gp else big
                pool = gpool if (on_gp or big_eng is not None) else ppool
                Pt = pool.tile([128, G, nrows, m - rs], dt, tag="P")
                if USE_ACT_UM and not on_gp:
                    UM = upool.tile([128, G, nrows, m - rs], dt, tag="UM")
                    nc.scalar.activation(
                        out=UM[:], in_=v_i,
                        func=mybir.ActivationFunctionType.Copy,
                    )
                    eng.tensor_tensor(out=Pt[:], in0=UM[:], in1=v_k, op=OP.mult)
                else:
                    eng.tensor_tensor(out=Pt[:], in0=v_i, in1=v_k, op=OP.mult)
                blk = A[:, :, j + 1 + rs : j + 1 + re, j + 1 + rs :]
                eng.tensor_tensor(out=blk, in0=blk, in1=Pt[:], op=OP.subtract)
            if b == 0:
                # row j+1 is final: issue the next step's recip + u now so
                # the cross-step chain doesn't wait on this step's tail.
                emit_recip_u(j + 1)

    # per-partition partial sum over g: sum_j log(p_j) + sum_j dcol_j^2/p_j
    a = A[:]
    diag = bass.AP(
        tensor=a.tensor, offset=a.offset,
        ap=[a.ap[0], [C * CP1, G], [CP1 + 1, C]],
    )
    LOGT = spool.tile([128, G, C], dt, tag="LOG")
    S = spool.tile([128, 1], F32, tag="S")
    nc.scalar.activation(
        out=LOGT[:], in_=diag, func=mybir.ActivationFunctionType.Ln,
        accum_out=S[:],
    )
    dcol = A[:, :, 0:C, C]                                  # final border col
    SQ = spool.tile([128, G, C], dt, tag="SQ")
    nc.vector.tensor_tensor(out=SQ[:], in0=dcol, in1=dcol, op=OP.mult)
    Q = spool.tile([128, 1], F32, tag="Q")
    nc.vector.scalar_tensor_tensor(
        out=SQ[:], in0=SQ[:], scalar=1.0, in1=RA[:], op0=OP.mult, op1=OP.mult,
        accum_out=Q[:],
    )
    V = spool.tile([128, 1], F32, tag="V")
    nc.vector.tensor_tensor(out=V[:], in0=S[:], in1=Q[:], op=OP.add)
    nc.sync.dma_start(out=vout[t * 128 : (t + 1) * 128, :], in_=V[:])


def _emit_tile_ginner(nc, pools, yp, vout, t, G):
    """g-innermost variant: one tile = G*128 problems stored interleaved as
    A[128, C, CP1, G] (fp16), i.e. element (i,k) of all G matrices adjacent.

    Every update operand then has innermost stride 1 over g (16 contiguous
    fp16 elements), so the outer-product mult, the subtract, AND the u scale
    all qualify for the DVE 2x_1p packed mode without materializing any
    broadcast operand -- the Scalar engine is not needed at all. The host
    pre-transposes y_pred/y_true to this layout so the load DMA stays dense.
    """
    apool, ppool, spool, _gpool, _upool = pools
    big = nc.vector
    dt = DT_A
    lo = t * 128
    A = apool.tile([128, C, CP1, G], dt, tag="A")
    # border column already holds d = y_true - mu (folded on the host).
    # Host also pre-casts to fp16, so the load is a same-dtype HWDGE copy
    # (hardware descriptor gen -- cheaper per DMA than SWDGE, Pool queue
    # stays free). Load in row-chunks so step-0 blocks start as soon as
    # their rows land (initial fill gap ~22us -> ~6us; Tile data deps do
    # the rest).
    rows_per = (C + LOAD_CHUNKS - 1) // LOAD_CHUNKS
    for r0 in range(0, C, rows_per):
        r1 = min(C, r0 + rows_per)
        nc.sync.dma_start(
            out=A[:, r0:r1, :, :],
            in_=yp[lo : lo + 128, r0 * CP1 * G : r1 * CP1 * G])
    RA = spool.tile([128, C, G], dt, tag="RA")            # 1/pivot per step

    U0 = spool.tile([128, C, G], dt, tag="U0")
    U1 = spool.tile([128, C, G], dt, tag="U1")
    Us = [U0, U1]

    neg_u = DMA_TAIL_FRAC > 0
    upd_op = OP.add if neg_u else OP.subtract

    def emit_recip_u(j):
        with nc.allow_low_precision(reason="per-pivot reciprocal, not an accum"):
            nc.vector.reciprocal(out=RA[:, j, :], in_=A[:, j, j, :])
        if neg_u:
            # RA holds -1/p: u comes out negated (updates become adds, the
            # accum-DMA's CCE only has add) and q's sign is fixed at the end.
            nc.vector.tensor_scalar(
                out=RA[:, j, :], in0=RA[:, j, :], scalar1=-1.0, scalar2=None,
                op0=OP.mult)
        if j == C - 1:
            return
        m = C - j
        big.tensor_tensor(
            out=Us[j % 2][:, 0:m, :], in0=A[:, j, j + 1 : CP1, :],
            in1=RA[:, j, None, :].broadcast_to([128, m, G]), op=OP.mult,
        )

    emit_recip_u(0)
    for j in range(C - 1):
        m = C - j
        U = Us[j % 2]
        row = A[:, j, j + 1 : CP1, :]                     # [128, m, G]
        mr = m - 1
        ts = mr  # first row of the DMA-offloaded tail block
        if DMA_TAIL_FRAC > 0 and m >= DMA_MIN_M:
            ts = int(mr * (1.0 - DMA_TAIL_FRAC))
            if mr - ts < 4:
                ts = mr
        if USE_DP_BOUNDS:
            bounds = _opt_bounds(ts, m)
        else:
            nb = 1 + sum(ts >= th for th in SPLIT_THRESHOLDS)
            bounds = [rs * ts // nb for rs in range(nb)] + [ts]
        if ts < mr:
            bounds = bounds + [mr]
        for b in range(len(bounds) - 1):
            rs, re = bounds[b], bounds[b + 1]
            nrows = re - rs
            v_i = U[:, rs:re, None, :].broadcast_to([128, nrows, m - rs, G])
            v_k = row[:, None, rs:m, :].broadcast_to([128, nrows, m - rs, G])
            Pt = ppool.tile([128, nrows, m - rs, G], dt, tag="P")
            big.tensor_tensor(out=Pt[:], in0=v_i, in1=v_k, op=OP.mult)
            blk = A[:, j + 1 + rs : j + 1 + re, j + 1 + rs :, :]
            if rs >= ts:
                nc.gpsimd.dma_start(out=blk, in_=Pt[:], accum_op=OP.add)
            else:
                big.tensor_tensor(out=blk, in0=blk, in1=Pt[:], op=upd_op)
            if b == 0:
                emit_recip_u(j + 1)

    # per-partition partial sum: sum_j log(p_j) + sum_j dcol_j^2 / p_j
    a = A[:]
    diag = bass.AP(
        tensor=a.tensor, offset=a.offset,
        ap=[a.ap[0], [(CP1 + 1) * G, C], [1, G]],
    )
    LOGT = spool.tile([128, C, G], dt, tag="LOG")
    S = spool.tile([128, 1], F32, tag="S")
    nc.scalar.activation(
        out=LOGT[:], in_=diag, func=mybir.ActivationFunctionType.Ln,
        accum_out=S[:],
    )
    dcol = A[:, 0:C, C, :]                                # final border col
    SQ = spool.tile([128, C, G], dt, tag="SQ")
    nc.vector.tensor_tensor(out=SQ[:], in0=dcol, in1=dcol, op=OP.mult)
    Q = spool.tile([128, 1], F32, tag="Q")
    nc.vector.scalar_tensor_tensor(
        out=SQ[:], in0=SQ[:], scalar=1.0, in1=RA[:], op0=OP.mult, op1=OP.mult,
        accum_out=Q[:],
    )
    V = spool.tile([128, 1], F32, tag="V")
    # with u negated, RA holds -1/p so Q = -q: compensate here
    nc.vector.tensor_tensor(
        out=V[:], in0=S[:], in1=Q[:], op=OP.subtract if neg_u else OP.add)
    nc.sync.dma_start(out=vout[t * 128 : (t + 1) * 128, :], in_=V[:])


def build3(ntiles: int, G: int) -> bass.Bass:
    """g-innermost packed variant (host pre-transposed inputs)."""
    nc = bacc.Bacc("TRN2", target_bir_lowering=False)
    yp = nc.dram_tensor("y_pred", [ntiles * 128, C * CP1 * G], F16,
                        kind="ExternalInput")
    vout = nc.dram_tensor("v_out", [ntiles * 128, 1], F32, kind="ExternalOutput")
    with TileContext(nc) as tc, ExitStack() as ctx:
        pools = _make_pools(tc, ctx, G)
        for t in range(ntiles):
            _emit_tile_ginner(nc, pools, yp, vout, t, G)
    if not nc.is_finalized():
        nc.finalize()
    return nc


def build_loop3(body_tiles: int, reps: int, G: int) -> bass.Bass:
    nc = bacc.Bacc("TRN2", target_bir_lowering=False)
    yp = nc.dram_tensor("y_pred", [body_tiles * 128, C * CP1 * G], F16,
                        kind="ExternalInput")
    vout = nc.dram_tensor("v_out", [body_tiles * 128, 1], F32,
                          kind="ExternalOutput")
    with TileContext(nc) as tc, ExitStack() as ctx:
        pools = _make_pools(tc, ctx, G)

        def body(i, unroll=1):
            for t in range(body_tiles):
                _emit_tile_ginner(nc, pools, yp, vout, t, G)

        with tc.For_i(0, reps, 1) as i:
            body(i)
    if not nc.is_finalized():
        nc.finalize()
    return nc


BORDER_IDX = np.arange(C) * CP1 + C


def to_ginner(ypf: np.ndarray, ytf: np.ndarray, ntiles: int, G: int):
    """Host-side relayout: per tile, problems (g, p) -> partition p holds the
    G matrices interleaved element-wise: row p = y_pred[(g,p), i, k] laid out
    as (i, k, g). The border column is replaced by d = y_true - mu so the
    device needs neither y_true nor the border subtract."""
    n = ntiles * G * 128
    yp = ypf[:n].copy()
    yp[:, BORDER_IDX] = ytf[:n] - yp[:, BORDER_IDX]
    # cast on host: the load DMA becomes a same-dtype HWDGE copy
    yp = yp.astype(np.float16)
    yp = yp.reshape(ntiles, G, 128, C * CP1)
    yp = yp.transpose(0, 2, 3, 1).reshape(ntiles * 128, C * CP1 * G)
    return np.ascontiguousarray(yp)


def build(ntiles: int = NTILES_FULL) -> bass.Bass:
    nprob = ntiles * 128
    nc = bacc.Bacc("TRN2", target_bir_lowering=False)
    yp = nc.dram_tensor("y_pred", [nprob, C * CP1], F32, kind="ExternalInput")
    yt = nc.dram_tensor("y_true", [nprob, C], F32, kind="ExternalInput")
    vout = nc.dram_tensor("v_out", [nprob, 1], F32, kind="ExternalOutput")

    with TileContext(nc) as tc, ExitStack() as ctx:
        pools = _make_pools(tc, ctx)
        for t in range(ntiles):
            _emit_tile(nc, pools, yp, yt, vout, t)
    if not nc.is_finalized():
        nc.finalize()
    return nc


def build_loop(body_tiles: int, reps: int) -> bass.Bass:
    """Timing amplifier: process the same `body_tiles` tiles `reps` times
    inside a For_i loop (static addressing; WAW across reps is fine)."""
    nprob = body_tiles * 128
    nc = bacc.Bacc("TRN2", target_bir_lowering=False)
    yp = nc.dram_tensor("y_pred", [nprob, C * CP1], F32, kind="ExternalInput")
    yt = nc.dram_tensor("y_true", [nprob, C], F32, kind="ExternalInput")
    vout = nc.dram_tensor("v_out", [nprob, 1], F32, kind="ExternalOutput")

    with TileContext(nc) as tc, ExitStack() as ctx:
        pools = _make_pools(tc, ctx)

        def body(i, unroll=1):
            for t in range(body_tiles):
                _emit_tile(nc, pools, yp, yt, vout, t)

        with tc.For_i(0, reps, 1) as i:
            body(i)
    if not nc.is_finalized():
        nc.finalize()
    return nc


def build2(ntiles: int, G: int) -> bass.Bass:
    """Packed variant: each tile covers G*128 problems."""
    nprob = ntiles * G * 128
    nc = bacc.Bacc("TRN2", target_bir_lowering=False)
    yp = nc.dram_tensor("y_pred", [nprob, C * CP1], F32, kind="ExternalInput")
    yt = nc.dram_tensor("y_true", [nprob, C], F32, kind="ExternalInput")
    vout = nc.dram_tensor("v_out", [ntiles * 128, 1], F32, kind="ExternalOutput")

    with TileContext(nc) as tc, ExitStack() as ctx:
        pools = _make_pools(tc, ctx, G)
        for t in range(ntiles):
            _emit_tile_packed(nc, pools, yp, yt, vout, t, G)
    if STRIP_DVE_WAITS:
        _strip_same_engine_dve_waits(nc)
    if not nc.is_finalized():
        nc.finalize()
    return nc


def build_loop2(body_tiles: int, reps: int, G: int, gp_every: int = 0,
                gp_tiles=()) -> bass.Bass:
    """gp_every=k: every k-th tile runs its big ops on GPSIMD (0 = never).
    gp_tiles: explicit tile indices to run on GPSIMD (overrides gp_every)."""
    nprob = body_tiles * G * 128
    nc = bacc.Bacc("TRN2", target_bir_lowering=False)
    yp = nc.dram_tensor("y_pred", [nprob, C * CP1], F32, kind="ExternalInput")
    yt = nc.dram_tensor("y_true", [nprob, C], F32, kind="ExternalInput")
    vout = nc.dram_tensor("v_out", [body_tiles * 128, 1], F32, kind="ExternalOutput")

    with TileContext(nc) as tc, ExitStack() as ctx:
        pools = _make_pools(tc, ctx, G)

        def body(i, unroll=1):
            for t in range(body_tiles):
                on_gp = (t in gp_tiles) or (gp_every and t % gp_every == gp_every - 1)
                eng = nc.gpsimd if on_gp else None
                _emit_tile_packed(nc, pools, yp, yt, vout, t, G, big_eng=eng)

        with tc.For_i(0, reps, 1) as i:
            body(i)
    if STRIP_DVE_WAITS:
        _strip_same_engine_dve_waits(nc)
    if not nc.is_finalized():
        nc.finalize()
    return nc


_CACHE: dict = {}


def _pad_rows(n_pad: int) -> tuple[np.ndarray, np.ndarray]:
    """Identity problems: Sigma=I, mu=0, y_true=0 -> v contribution exactly 0."""
    row = np.concatenate([np.eye(C, dtype=np.float32), np.zeros((C, 1), np.float32)], axis=1)
    return (
        np.tile(row.reshape(1, -1), (n_pad, 1)),
        np.zeros((n_pad, C), np.float32),
    )


G_PACK = 16
NTILES_PACKED = PAD // (G_PACK * 128)  # 2


def kernel(y_true: np.ndarray, y_pred: np.ndarray) -> np.ndarray:
    # np.asarray also handles jax arrays (device -> host copy)
    ypf = np.ascontiguousarray(
        np.asarray(y_pred, dtype=np.float32).reshape(NPROB, C * CP1))
    ytf = np.ascontiguousarray(
        np.asarray(y_true, dtype=np.float32).reshape(NPROB, C))

    if "nc" not in _CACHE:
        _CACHE["nc"] = build3(NTILES_PACKED, G_PACK)
    nc = _CACHE["nc"]

    pad_p, pad_t = _pad_rows(PAD - PER_CORE)
    in_maps = []
    for c in range(N_CORES):
        sl = slice(c * PER_CORE, (c + 1) * PER_CORE)
        ypg = to_ginner(
            np.concatenate([ypf[sl], pad_p], axis=0),
            np.concatenate([ytf[sl], pad_t], axis=0),
            NTILES_PACKED, G_PACK,
        )
        in_maps.append({"y_pred": ypg})

    # Transient device flakes (observed ~once per dozen runs) can yield NaN;
    # the result is cheap to validate, so retry a couple of times on
    # non-finite output before giving up.
    for _attempt in range(3):
        res = run_bass_kernel_spmd(nc, in_maps, core_ids=list(range(N_CORES)))
        # v_out rows are per-partition partial sums (padding contributes 0)
        v = np.concatenate([r["v_out"][:, 0] for r in res.results])
        loss = 0.5 * float(np.sum(v, dtype=np.float64)) / B + T * 0.5 * C * LOG_2PI
        if np.isfinite(loss):
            break
    return np.float32(loss)

